# revision 2
# baseline (speedup 1.0000x reference)
"""MLA forward kernel for Trainium2, 8 NeuronCores — fp8 comp3 edition.

Sharding: 8 cores = 2 (batch) x 4 (head-groups of 10 heads), as baseline.

vs baseline:
  - A/B/C/E matmuls use fp8e4 DoubleRow "comp3": x ~ xh+xl, w ~ wh+wl (both
    e4m3, weights pre-scaled x32 on host), psum = xh@wh + xl@wh + xh@wl.
    Each DoubleRow instruction contracts 2 k-tiles at 0.5 cycles/row.
  - scores/AV stay bf16 (accuracy + layout simplicity).
  - activations gathered across cores as fp8 hi/lo; q gather split into a
    hi gather and a lo gather, with phase C split into a hi-pass (2 terms,
    partial qT in DRAM) and a lo-pass (1 term + merge) to shrink the
    collective exposure window.
  - attention out osb stored as fp8 hi/lo at x4 scale; o-proj is comp3;
    host divides the final sum by 128 (= 4 * 32).
  - exp batched 2-wide over score k-tiles ([128,1024] per Act instruction).
"""

import math
import sys
from dataclasses import dataclass

if "/opt/trn_rl_repo" not in sys.path:
    sys.path.insert(0, "/opt/trn_rl_repo")

import ml_dtypes
import numpy as np

BF16 = ml_dtypes.bfloat16
E4M3 = ml_dtypes.float8_e4m3
WS = 32.0          # host weight pre-scale (power of 2)
OS = 4.0           # osb scale (via 0.25 ones column)


@dataclass(frozen=True)
class Cfg:
    HID: int = 5120
    S: int = 2048
    QLR: int = 1536
    KVLR: int = 512
    DN: int = 128
    DR: int = 64
    DV: int = 128
    HPC: int = 10
    CHUNK: int = 512
    GS: int = 1
    NCORES: int = 8
    EPS: float = 1e-6
    THETA: float = 10000.0

    @property
    def DQK(self):
        return self.DN + self.DR

    @property
    def PEH(self):
        return self.DR // 2

    @property
    def SL(self):
        return self.S // self.GS


FULL = Cfg(GS=4)
DR_MODE = True  # DoubleRow fp8


def build_program(c: Cfg, stop_after: str = "E"):
    import contextlib

    import concourse.bass as bass  # noqa: F401
    import concourse.mybir as mybir
    import concourse.tile as tile
    from concourse import bacc
    from concourse.masks import make_identity

    dt = mybir.dt
    BF = dt.bfloat16
    F32 = dt.float32
    FP8 = dt.float8e4
    Alu = mybir.AluOpType
    Act = mybir.ActivationFunctionType
    DRow = mybir.MatmulPerfMode.DoubleRow

    KT_HID = c.HID // 128
    KT_Q = c.QLR // 128
    KT_KV = c.KVLR // 128
    NQC = c.S // c.CHUNK
    GS = c.GS
    SL = c.SL
    ST = c.S // 128
    H = c.HPC
    TPC = c.CHUNK // 128
    MT_QN = H * c.DN // 128
    MT_QP = H * c.DR // 128
    QROWS = H * (c.DN + c.DR)
    KROWS = H * c.DN
    VCOLS = H * c.DV
    MT_O = c.HID // 128
    MT_A = KT_Q + KT_KV + 1
    SCALE = 1.0 / math.sqrt(c.DQK)
    EV = SCALE / WS   # eviction scale for q tiles
    IV = 1.0 / WS     # eviction scale for a/kv tiles

    assert c.DN == 128 and c.DV == 128 and c.DR == 64 and H % 2 == 0
    assert KT_HID % 2 == 0 and KT_Q % 2 == 0 and KT_KV % 2 == 0
    _PH = ["A", "B", "C", "D", "E"]
    enabled = set(_PH[:_PH.index(stop_after) + 1])

    nc = bacc.Bacc("TRN2", num_devices=(c.NCORES if GS > 1 else None))
    xh = nc.dram_tensor("xh", [128, KT_HID * SL], FP8, kind="ExternalInput")
    xl = nc.dram_tensor("xl", [128, KT_HID * SL], FP8, kind="ExternalInput")
    wa_h = nc.dram_tensor("wa_h", [128, MT_A * KT_HID * 128], FP8,
                          kind="ExternalInput")
    wa_l = nc.dram_tensor("wa_l", [128, MT_A * KT_HID * 128], FP8,
                          kind="ExternalInput")
    wqb_hl = nc.dram_tensor(
        "wqb_hl", [128, (MT_QN + MT_QP) * 2 * KT_Q * 128], FP8,
        kind="ExternalInput")
    wkv_h = nc.dram_tensor("wkv_h", [128, KT_KV * (KROWS + VCOLS)], FP8,
                           kind="ExternalInput")
    wkv_l = nc.dram_tensor("wkv_l", [128, KT_KV * (KROWS + VCOLS)], FP8,
                           kind="ExternalInput")
    wo_hl = nc.dram_tensor("wo_hl", [128, MT_O * 2 * H * 128], FP8,
                           kind="ExternalInput")
    cosT = nc.dram_tensor("cosT", [128, c.S], BF, kind="ExternalInput")
    sinT = nc.dram_tensor("sinT", [128, c.S], BF, kind="ExternalInput")
    cosA = nc.dram_tensor("cosA", [128, SL], BF, kind="ExternalInput")
    sinA = nc.dram_tensor("sinA", [128, SL], BF, kind="ExternalInput")
    lnq = nc.dram_tensor("lnq", [128, KT_Q], F32, kind="ExternalInput")
    lnkv = nc.dram_tensor("lnkv", [128, KT_KV], F32, kind="ExternalInput")
    maskm = nc.dram_tensor("maskm", [128, TPC, c.CHUNK], BF,
                           kind="ExternalInput")
    outT = nc.dram_tensor("outT", [c.HID, c.S], F32, kind="ExternalOutput")
    qTs = nc.dram_tensor("qTs", [QROWS, c.S], BF, kind="Internal")
    qTs8 = nc.dram_tensor("qTs8", [H * 512, c.S], FP8, kind="Internal")
    # gather buffers: kv hi(4kt)+lo(4kt) fp8; kpe bf16; q hi / q lo fp8
    aglkv = nc.dram_tensor("aglkv", [2 * KT_KV * 128, SL], FP8,
                           kind="Internal")
    aglpe = nc.dram_tensor("aglpe", [c.DR, SL], BF, kind="Internal")
    aglq = nc.dram_tensor("aglq", [2 * KT_Q * 128, SL], FP8, kind="Internal")
    if GS > 1:
        aggkv = nc.dram_tensor("aggkv", [GS * 2 * KT_KV * 128, SL], FP8,
                               kind="Internal")
        aggpe = nc.dram_tensor("aggpe", [GS * c.DR, SL], BF, kind="Internal")
        aggq = nc.dram_tensor("aggq", [GS * 2 * KT_Q * 128, SL], FP8,
                              kind="Internal")
    else:
        aggkv, aggpe, aggq = aglkv, aglpe, aglq

    xh_r = xh.ap().rearrange("p (t s) -> p t s", s=SL)
    xl_r = xl.ap().rearrange("p (t s) -> p t s", s=SL)
    wa_h_r = wa_h.ap().rearrange("p (mt k m) -> p mt (k m)", mt=MT_A, m=128)
    wa_l_r = wa_l.ap().rearrange("p (mt k m) -> p mt (k m)", mt=MT_A, m=128)
    wqb_hl_r = wqb_hl.ap().rearrange("p (mt t k m) -> p mt (t k m)",
                                     mt=MT_QN + MT_QP, m=128, t=2)
    wkv_h_r = wkv_h.ap().rearrange("p (k m) -> p k m", k=KT_KV)
    wkv_l_r = wkv_l.ap().rearrange("p (k m) -> p k m", k=KT_KV)
    wo_hl_r = wo_hl.ap().rearrange("p (mt t k m) -> p mt (t k m)",
                                   mt=MT_O, m=128, t=2)
    aggkv_r = aggkv.ap().rearrange("(g t p) s -> g p t s", g=GS, p=128)
    aggpe_r = aggpe.ap().rearrange("(g p) s -> g p s", g=GS)
    aggq_r = aggq.ap().rearrange("(g t p) s -> g p t s", g=GS, p=128)
    aglkv_r = aglkv.ap().rearrange("(t p) s -> p t s", p=128)
    aglq_r = aglq.ap().rearrange("(t p) s -> p t s", p=128)
    qTs_ap = qTs.ap()
    qTs8_r = qTs8.ap().rearrange("(h q) s -> h q s", q=512)
    outT_ap = outT.ap()
    groups = [[b * GS + j for j in range(GS)] for b in range(c.NCORES // GS)]

    def dr_terms(ps, wh_t, wl_t, xh_t, xl_t, nk, msl, hi_only=False,
                 lo_only=False, start=True, stop=True):
        """Emit comp3 DoubleRow matmuls into psum `ps`.

        wh_t/wl_t: [128, nk, M] fp8 SBUF; xh_t/xl_t: [128, nk, N].
        msl: slice on the M dim of the weight tiles.
        """
        terms = []
        if not lo_only:
            terms += [(wh_t, xh_t), (wl_t, xh_t)]
        if not hi_only:
            terms += [(wh_t, xl_t)]
        np_ = nk // 2
        n_ins = len(terms) * np_
        i = 0
        for (wt, xt) in terms:
            for j in range(np_):
                nc.tensor.matmul(
                    ps, wt[:, 2 * j:2 * j + 2, msl],
                    xt[:, 2 * j:2 * j + 2, :],
                    start=(start and i == 0),
                    stop=(stop and i == n_ins - 1),
                    perf_mode=DRow)
                i += 1

    def emit_rope(pool, dst64, src64, cos_ap, sin_ap, W, p0=0, ph=None):
        ph = ph or c.PEH
        t1, t2 = src64[0:ph], src64[ph:2 * ph]
        d1, d2 = dst64[0:ph], dst64[ph:2 * ph]
        c1, s1 = cos_ap[p0:p0 + ph], sin_ap[p0:p0 + ph]
        c2, s2 = cos_ap[p0 + ph:p0 + 2 * ph], sin_ap[p0 + ph:p0 + 2 * ph]
        ra = pool.tile([ph, W], BF, tag="rope_a", name="rope_a")
        rb = pool.tile([ph, W], BF, tag="rope_b", name="rope_b")
        nc.vector.tensor_tensor(out=ra, in0=t1, in1=c1, op=Alu.mult)
        nc.vector.tensor_tensor(out=rb, in0=t2, in1=s2, op=Alu.mult)
        nc.vector.tensor_tensor(out=d1, in0=ra, in1=rb, op=Alu.subtract)
        nc.vector.tensor_tensor(out=ra, in0=t2, in1=c2, op=Alu.mult)
        nc.vector.tensor_tensor(out=rb, in0=t1, in1=s1, op=Alu.mult)
        nc.vector.tensor_tensor(out=d2, in0=ra, in1=rb, op=Alu.add)

    with tile.TileContext(nc, pool_alloc_mode="queue") as tc:
        with contextlib.ExitStack() as top:
            pers = top.enter_context(tc.tile_pool(name="pers", bufs=1))
            cos_sb = pers.tile([128, c.S], BF, tag="cos_sb")
            sin_sb = pers.tile([128, c.S], BF, tag="sin_sb")
            cosa_sb = pers.tile([128, SL], BF, tag="cosa_sb")
            sina_sb = pers.tile([128, SL], BF, tag="sina_sb")
            lnq_sb = pers.tile([128, KT_Q], F32, tag="lnq_sb")
            lnkv_sb = pers.tile([128, KT_KV], F32, tag="lnkv_sb")
            mask_sb = pers.tile([128, TPC, c.CHUNK], BF, tag="mask_sb")
            ident = pers.tile([128, 128], BF, tag="ident")
            ones_f = pers.tile([1, 128], F32, tag="ones_f")
            ones_c = pers.tile([128, 1], BF, tag="ones_c")
            eps_sb = pers.tile([1, 1], F32, tag="eps_sb")
            nc.vector.memset(eps_sb, c.EPS)
            kpe = pers.tile([c.DR, c.S], BF, tag="kpe")
            nc.sync.dma_start(out=cos_sb, in_=cosT.ap())
            nc.sync.dma_start(out=sin_sb, in_=sinT.ap())
            nc.sync.dma_start(out=cosa_sb, in_=cosA.ap())
            nc.sync.dma_start(out=sina_sb, in_=sinA.ap())
            nc.sync.dma_start(out=lnq_sb, in_=lnq.ap())
            nc.sync.dma_start(out=lnkv_sb, in_=lnkv.ap())
            nc.sync.dma_start(out=mask_sb, in_=maskm.ap())
            make_identity(nc, ident)
            nc.vector.memset(ones_f, 1.0)
            nc.vector.memset(ones_c, 1.0)

            # -------- phase A ------------------------------------------------
            with contextlib.ExitStack() as st:
                pax = st.enter_context(tc.tile_pool(name="pax", bufs=1))
                paw = st.enter_context(tc.tile_pool(name="paw", bufs=3))
                pat = st.enter_context(tc.tile_pool(name="pat", bufs=2))
                paa = st.enter_context(tc.tile_pool(name="paa", bufs=1))
                paps = st.enter_context(
                    tc.tile_pool(name="paps", bufs=3, space="PSUM"))
                pssq = st.enter_context(
                    tc.tile_pool(name="pssq", bufs=1, space="PSUM"))
                pbc = st.enter_context(
                    tc.tile_pool(name="pbc", bufs=2, space="PSUM"))

                xh_sb = pax.tile([128, KT_HID, SL], FP8, tag="xh_sb")
                xl_sb = pax.tile([128, KT_HID, SL], FP8, tag="xl_sb")
                nc.sync.dma_start(out=xh_sb, in_=xh_r)
                nc.sync.dma_start(out=xl_sb, in_=xl_r)
                aq_c = paa.tile([128, KT_Q, SL], BF, tag="aq_c")
                akv_c = paa.tile([128, KT_KV, SL], BF, tag="akv_c")
                aq8h = paa.tile([128, KT_Q, SL], FP8, tag="aq8h")
                aq8l = paa.tile([128, KT_Q, SL], FP8, tag="aq8l")
                akv8 = paa.tile([128, 2 * KT_KV, SL], FP8, tag="akv8")
                ssq_q = pssq.tile([1, SL], F32, tag="ssq_q")
                ssq_kv = pssq.tile([1, SL], F32, tag="ssq_kv")
                kperaw = pat.tile([c.DR, SL], BF, tag="kperaw", bufs=1)
                kpel = pat.tile([c.DR, SL], BF, tag="kpel", bufs=1)

                def normalize(ssq, ln_sb, ktn, denom, dst):
                    rn = pat.tile([1, c.CHUNK], F32, tag="rn", name="rn")
                    nc.scalar.activation(
                        rn, ssq, Act.Sqrt, bias=eps_sb, scale=1.0 / denom)
                    rnr = pat.tile([1, c.CHUNK], F32, tag="rnr", name="rnr")
                    nc.vector.reciprocal(rnr, rn)
                    bc = pbc.tile([128, c.CHUNK], F32, tag="bc", name="bc")
                    nc.tensor.matmul(bc, ones_f, rnr, start=True, stop=True)
                    for t in range(ktn):
                        tgt = dst[:, t, :]
                        nc.vector.scalar_tensor_tensor(
                            out=tgt, in0=tgt, scalar=ln_sb[:, t:t + 1],
                            in1=bc, op0=Alu.mult, op1=Alu.mult)

                mtiles = ([("kv", i) for i in range(KT_KV)]
                          + [("pe", 0)]
                          + [("q", i) for i in range(KT_Q)])
                for mti, (seg, ti) in enumerate(mtiles):
                    mw = c.DR if seg == "pe" else 128
                    wah_sb = paw.tile([128, KT_HID, 128], FP8, tag="wah",
                                      name="wah")
                    wal_sb = paw.tile([128, KT_HID, 128], FP8, tag="wal",
                                      name="wal")
                    nc.sync.dma_start(
                        out=wah_sb,
                        in_=wa_h_r[:, mti].rearrange("p (k m) -> p k m",
                                                     m=128))
                    nc.sync.dma_start(
                        out=wal_sb,
                        in_=wa_l_r[:, mti].rearrange("p (k m) -> p k m",
                                                     m=128))
                    ps = paps.tile([128, c.CHUNK], F32, tag="aps", name="ps")
                    dr_terms(ps[:mw], wah_sb, wal_sb, xh_sb, xl_sb,
                             KT_HID, slice(0, mw))
                    if seg == "q":
                        nc.scalar.activation(aq_c[:, ti, :], ps, Act.Copy,
                                             scale=IV)
                        sq = pat.tile([128, c.CHUNK], BF, tag="sq", bufs=3,
                                      name="sq")
                        nc.scalar.activation(sq, ps, Act.Square, scale=IV)
                        nc.tensor.matmul(
                            ssq_q, ones_c, sq,
                            start=(ti == 0), stop=(ti == KT_Q - 1))
                    elif seg == "kv":
                        nc.scalar.activation(akv_c[:, ti, :], ps, Act.Copy,
                                             scale=IV)
                        sq = pat.tile([128, c.CHUNK], BF, tag="sq", bufs=3,
                                      name="sq")
                        nc.scalar.activation(sq, ps, Act.Square, scale=IV)
                        nc.tensor.matmul(
                            ssq_kv, ones_c, sq,
                            start=(ti == 0), stop=(ti == KT_KV - 1))
                    else:
                        nc.scalar.activation(kperaw, ps[:mw], Act.Copy,
                                             scale=IV)
                    if seg == "pe":
                        # kv part complete: normalize, rope, split, gather
                        normalize(ssq_kv, lnkv_sb, KT_KV, c.KVLR, akv_c)
                        emit_rope(pat, kpel, kperaw, cosa_sb, sina_sb,
                                  c.CHUNK)
                        for t in range(KT_KV):
                            nc.scalar.copy(akv8[:, t, :], akv_c[:, t, :])
                            nc.vector.scalar_tensor_tensor(
                                out=akv8[:, KT_KV + t, :],
                                in0=akv_c[:, t, :], scalar=1.0,
                                in1=akv8[:, t, :],
                                op0=Alu.mult, op1=Alu.subtract)
                        nc.sync.dma_start(out=aglkv_r, in_=akv8)
                        nc.sync.dma_start(out=aglpe.ap(), in_=kpel)
                        if GS > 1:
                            nc.gpsimd.collective_compute(
                                "AllGather", Alu.bypass,
                                replica_groups=groups,
                                ins=[aglkv.ap()], outs=[aggkv.ap()])
                            nc.gpsimd.collective_compute(
                                "AllGather", Alu.bypass,
                                replica_groups=groups,
                                ins=[aglpe.ap()], outs=[aggpe.ap()])
                normalize(ssq_q, lnq_sb, KT_Q, c.QLR, aq_c)
                for t in range(KT_Q):
                    nc.scalar.copy(aq8h[:, t, :], aq_c[:, t, :])
                    nc.vector.scalar_tensor_tensor(
                        out=aq8l[:, t, :], in0=aq_c[:, t, :], scalar=1.0,
                        in1=aq8h[:, t, :], op0=Alu.mult, op1=Alu.subtract)
                nc.sync.dma_start(out=aglq_r[:, 0:KT_Q, :], in_=aq8h)
                nc.sync.dma_start(out=aglq_r[:, KT_Q:2 * KT_Q, :], in_=aq8l)
                if GS > 1:
                    nc.gpsimd.collective_compute(
                        "AllGather", Alu.bypass, replica_groups=groups,
                        ins=[aglq.ap()], outs=[aggq.ap()])

            for g in range(GS):
                nc.sync.dma_start(out=kpe[:, g * SL:(g + 1) * SL],
                                  in_=aggpe_r[g])

            # -------- phase B: kv up-projection ------------------------------
            if "B" in enabled:
                pkv = top.enter_context(tc.tile_pool(name="pkv", bufs=1))
                kn = pkv.tile([128, H, 2, c.S], FP8, tag="kn")
                vv = pkv.tile([128, ST, H, c.DV + 1], BF, tag="vv")
                nc.vector.memset(vv[:, :, :, c.DV:], 1.0 / OS)
                kpe8 = pkv.tile([c.DR, 2, c.S], FP8, tag="kpe8")
                nc.scalar.copy(kpe8[:, 0, :], kpe)
                nc.vector.scalar_tensor_tensor(
                    out=kpe8[:, 1, :], in0=kpe, scalar=1.0,
                    in1=kpe8[:, 0, :], op0=Alu.mult, op1=Alu.subtract)

                with contextlib.ExitStack() as st:
                    pbw = st.enter_context(tc.tile_pool(name="pbw", bufs=1))
                    pbps = st.enter_context(
                        tc.tile_pool(name="pbps", bufs=3, space="PSUM"))
                    wkvh_sb = pbw.tile([128, KT_KV, KROWS + VCOLS], FP8,
                                       tag="wkvh")
                    wkvl_sb = pbw.tile([128, KT_KV, KROWS + VCOLS], FP8,
                                       tag="wkvl")
                    nc.sync.dma_start(out=wkvh_sb, in_=wkv_h_r)
                    nc.sync.dma_start(out=wkvl_sb, in_=wkv_l_r)
                    akvfh = pbw.tile([128, KT_KV, c.S], FP8, tag="akvfh")
                    akvfl = pbw.tile([128, KT_KV, c.S], FP8, tag="akvfl")
                    for g in range(GS):
                        sl_ = slice(g * SL, (g + 1) * SL)
                        nc.sync.dma_start(out=akvfh[:, :, sl_],
                                          in_=aggkv_r[g, :, 0:KT_KV, :])
                        nc.sync.dma_start(out=akvfl[:, :, sl_],
                                          in_=aggkv_r[g, :, KT_KV:2 * KT_KV, :])
                    ev_flip = [0]

                    def evict(dst, src, scale):
                        # alternate Act / DVE to balance engine load
                        if ev_flip[0] % 2 == 0:
                            nc.scalar.activation(dst, src, Act.Copy,
                                                 scale=scale)
                        else:
                            nc.vector.tensor_scalar_mul(dst, src, scale)
                        ev_flip[0] += 1

                    for mt in range(H):
                        msl = slice(mt * 128, (mt + 1) * 128)
                        for qc in range(NQC):
                            cs = slice(qc * c.CHUNK, (qc + 1) * c.CHUNK)
                            ps = pbps.tile([128, c.CHUNK], F32, tag="kps")
                            dr_terms(ps, wkvh_sb, wkvl_sb,
                                     akvfh[:, :, cs], akvfl[:, :, cs],
                                     KT_KV, msl)
                            nc.scalar.activation(kn[:, mt, 0, cs], ps,
                                                 Act.Copy, scale=IV)
                            nc.vector.scalar_tensor_tensor(
                                out=kn[:, mt, 1, cs], in0=ps, scalar=IV,
                                in1=kn[:, mt, 0, cs],
                                op0=Alu.mult, op1=Alu.subtract)
                    vch = []
                    v0 = 0
                    while v0 < VCOLS:
                        vw = min(512, VCOLS - v0)
                        vch.append((v0, vw))
                        v0 += vw
                    for stt_ in range(ST):
                        ss = slice(stt_ * 128, (stt_ + 1) * 128)
                        for v0, vw in vch:
                            vsl = slice(KROWS + v0, KROWS + v0 + vw)
                            ps = pbps.tile([128, 512], F32, tag="vps")
                            dr_terms(ps[:, :vw], akvfh[:, :, ss],
                                     akvfl[:, :, ss], wkvh_sb[:, :, vsl],
                                     wkvl_sb[:, :, vsl], KT_KV,
                                     slice(0, 128))
                            h0, hn = v0 // c.DV, vw // c.DV
                            evict(
                                vv[:, stt_, h0:h0 + hn, 0:c.DV],
                                ps[:, :vw].rearrange("p (h d) -> p h d",
                                                     d=c.DV), IV)

            # -------- phase C: q up-projection (single comp3 pass) -----------
            if "C" in enabled:
                with contextlib.ExitStack() as st:
                    pcq = st.enter_context(tc.tile_pool(name="pcq", bufs=1))
                    pcw = st.enter_context(tc.tile_pool(name="pcw", bufs=3))
                    pce = st.enter_context(tc.tile_pool(name="pce", bufs=3))
                    pcps = st.enter_context(
                        tc.tile_pool(name="pcps", bufs=4, space="PSUM"))
                    aqfh = pcq.tile([128, KT_Q, c.S], FP8, tag="aqfh")
                    aqfl = pcq.tile([128, KT_Q, c.S], FP8, tag="aqfl")
                    for g in range(GS):
                        sl_ = slice(g * SL, (g + 1) * SL)
                        nc.sync.dma_start(out=aqfh[:, :, sl_],
                                          in_=aggq_r[g, :, 0:KT_Q, :])
                        nc.sync.dma_start(out=aqfl[:, :, sl_],
                                          in_=aggq_r[g, :, KT_Q:2 * KT_Q, :])
                    mt_order = []
                    for j in range(MT_QP):
                        mt_order += [2 * j, 2 * j + 1, MT_QN + j]
                    mt_order += list(range(2 * MT_QP, MT_QN))
                    for mt in mt_order:
                        wq_sb = pcw.tile([128, 2, KT_Q, 128], FP8, tag="wq")
                        nc.sync.dma_start(
                            out=wq_sb,
                            in_=wqb_hl_r[:, mt].rearrange(
                                "p (t k m) -> p t k m", t=2, m=128))
                        for qc in range(NQC):
                            col = qc * c.CHUNK
                            cs = slice(col, col + c.CHUNK)
                            ps = pcps.tile([128, c.CHUNK], F32, tag="qps")
                            dr_terms(ps, wq_sb[:, 0], wq_sb[:, 1],
                                     aqfh[:, :, cs], aqfl[:, :, cs],
                                     KT_Q, slice(0, 128))
                            if mt >= MT_QN:
                                qsb = pce.tile([128, c.CHUNK], BF, tag="qsb")
                                nc.scalar.activation(qsb, ps, Act.Copy,
                                                     scale=IV)
                                roped = pce.tile([128, c.CHUNK], FP8,
                                                 tag="roped")
                                emit_rope(pce, roped, qsb,
                                          cos_sb[:, cs], sin_sb[:, cs],
                                          c.CHUNK, ph=64)
                                j2 = mt - MT_QN
                                for sl8 in (256, 384):
                                    for j in (0, 1):
                                        dst = qTs8_r[
                                            2 * j2:2 * j2 + 2,
                                            sl8 + j * 32:sl8 + j * 32 + 32,
                                            cs]
                                        nc.sync.dma_start(
                                            out=dst,
                                            in_=roped[j * 64:(j + 1) * 64])
                            else:
                                qf8 = pce.tile([128, c.CHUNK], FP8,
                                               tag="qf8")
                                nc.scalar.activation(qf8, ps, Act.Copy,
                                                     scale=IV)
                                dst = qTs8_r[mt, 0:256, cs].rearrange(
                                    "(two p) s -> p two s", p=128)
                                nc.sync.dma_start(
                                    out=dst,
                                    in_=qf8[:, None, :].broadcast_to(
                                        [128, 2, c.CHUNK]))
            # -------- phase D: attention -------------------------------------
            if "D" in enabled:
                pot = top.enter_context(tc.tile_pool(name="pot", bufs=1))
                oT_h = pot.tile([128, H, c.S], FP8, tag="oT_h")
                oT_l = pot.tile([128, H, c.S], FP8, tag="oT_l")

                with contextlib.ExitStack() as st:
                    pdp = st.enter_context(tc.tile_pool(name="pdp", bufs=2))
                    pdq = st.enter_context(tc.tile_pool(name="pdq", bufs=3))
                    pde = st.enter_context(tc.tile_pool(name="pde", bufs=6))
                    pew = st.enter_context(tc.tile_pool(name="pew", bufs=3))
                    peo = st.enter_context(tc.tile_pool(name="peo", bufs=3))
                    pds = st.enter_context(
                        tc.tile_pool(name="pds", bufs=2, space="PSUM"))
                    pdo = st.enter_context(
                        tc.tile_pool(name="pdo", bufs=1, space="PSUM"))
                    pdt = st.enter_context(
                        tc.tile_pool(name="pdt", bufs=1, space="PSUM"))
                    peps = st.enter_context(
                        tc.tile_pool(name="peps", bufs=2, space="PSUM"))

                    def e_tiles(qc, mts):
                        cs = slice(qc * c.CHUNK, (qc + 1) * c.CHUNK)
                        for mt in mts:
                            wo_sb = pew.tile([128, 2, H, 128], FP8, tag="wo")
                            nc.sync.dma_start(
                                out=wo_sb,
                                in_=wo_hl_r[:, mt].rearrange(
                                    "p (t k m) -> p t k m", t=2, m=128))
                            ps = peps.tile([128, c.CHUNK], F32, tag="ops")
                            dr_terms(ps, wo_sb[:, 0], wo_sb[:, 1],
                                     oT_h[:, :, cs],
                                     oT_l[:, :, cs], H, slice(0, 128))
                            ob = peo.tile([128, c.CHUNK], F32, tag="ob")
                            if mt % 2 == 0:
                                nc.scalar.copy(ob, ps)
                            else:
                                nc.vector.tensor_copy(out=ob, in_=ps)
                            nc.sync.dma_start(
                                out=outT_ap[mt * 128:(mt + 1) * 128, cs],
                                in_=ob)

                    def d_chunk(qc, eq):
                        col = qc * c.CHUNK
                        kmax = min(TPC * qc + TPC, ST)
                        for h in range(H):
                            qf = pdq.tile([128, 4, c.CHUNK], FP8, tag="qf")
                            nc.sync.dma_start(
                                out=qf,
                                in_=qTs8_r[h, :, col:col + c.CHUNK].rearrange(
                                    "(fo p) s -> p fo s", p=128))
                            probs = pdp.tile([128, ST, c.CHUNK], BF,
                                             tag="probs")
                            for k2 in range(kmax // 2):
                                ps = pds.tile([128, 2, c.CHUNK], F32,
                                              tag="sc")
                                for i in (0, 1):
                                    kt = 2 * k2 + i
                                    ksl = slice(kt * 128, (kt + 1) * 128)
                                    nc.tensor.matmul(
                                        ps[:, i, :], kn[:, h, :, ksl],
                                        qf[:, 0:2, :], start=True,
                                        stop=False, perf_mode=DRow)
                                    nc.tensor.matmul(
                                        ps[:, i, :], kpe8[:, :, ksl],
                                        qf[0:64, 2:4, :], start=False,
                                        stop=True, perf_mode=DRow)
                                nc.scalar.activation(
                                    probs[:, 2 * k2:2 * k2 + 2, :], ps,
                                    Act.Exp, scale=SCALE)
                                for i in (0, 1):
                                    kt = 2 * k2 + i
                                    d = kt - TPC * qc
                                    if d >= 0:
                                        nc.vector.tensor_tensor(
                                            out=probs[:, kt, :],
                                            in0=probs[:, kt, :],
                                            in1=mask_sb[:, d, :],
                                            op=Alu.mult)
                            for q2 in range(TPC):
                                qt = TPC * qc + q2
                                po = pdo.tile([128, c.DV + 1], F32, tag="po")
                                for kt in range(qt + 1):
                                    nc.tensor.matmul(
                                        po,
                                        probs[:, kt, q2 * 128:(q2 + 1) * 128],
                                        vv[:, kt, h, :],
                                        start=(kt == 0), stop=(kt == qt))
                                rec = pde.tile([128, 1], F32, tag="rec")
                                nc.vector.reciprocal(
                                    rec, po[:, c.DV:c.DV + 1])
                                osb = pde.tile([128, c.DV], BF, tag="osb")
                                nc.vector.tensor_scalar_mul(
                                    osb, po[:, :c.DV], rec)
                                pt = pdt.tile([128, 128], BF, tag="pt")
                                nc.tensor.transpose(pt, osb, ident)
                                ql_ = slice(qt * 128, (qt + 1) * 128)
                                nc.vector.tensor_copy(
                                    out=oT_h[:, h, ql_], in_=pt)
                                nc.vector.scalar_tensor_tensor(
                                    out=oT_l[:, h, ql_], in0=pt, scalar=1.0,
                                    in1=oT_h[:, h, ql_],
                                    op0=Alu.mult, op1=Alu.subtract)
                            if eq is not None and "E" in enabled:
                                e_tiles(eq, range(4 * h, 4 * h + 4))

                    d_chunk(0, None)
                    d_chunk(1, 0)
                    d_chunk(2, 1)
                    d_chunk(3, 2)
                    if "E" in enabled:
                        e_tiles(3, range(MT_O))

    nc.compile()
    return nc


# ---------------------------------------------------------------------------
# host-side input preparation
# ---------------------------------------------------------------------------

def split8(x):
    hi = np.asarray(x, np.float32).astype(E4M3)
    lo = (np.asarray(x, np.float32) - hi.astype(np.float32)).astype(E4M3)
    return hi, lo


def prep_shared(c: Cfg, w_a, q_ln_w, kv_ln_w):
    KT_Q = c.QLR // 128
    KT_KV = c.KVLR // 128
    TPC = c.CHUNK // 128
    half = c.PEH
    inv_freq = 1.0 / (c.THETA ** (np.arange(half, dtype=np.float32) / half))
    ang = np.arange(c.S, dtype=np.float32)[:, None] * inv_freq[None, :]
    cosT = np.ascontiguousarray(
        np.tile(np.cos(ang).T, (128 // half, 1))).astype(BF16)
    sinT = np.ascontiguousarray(
        np.tile(np.sin(ang).T, (128 // half, 1))).astype(BF16)
    k_idx = np.arange(128)[:, None]
    q_idx = np.arange(c.CHUNK)[None, :]
    maskm = np.stack(
        [(k_idx <= q_idx - 128 * d) for d in range(TPC)], axis=1
    ).astype(BF16)
    MT_A = KT_Q + KT_KV + 1
    KT_HID = c.HID // 128
    wa = np.asarray(w_a, np.float32) * WS
    order = ([c.QLR + i * 128 for i in range(KT_KV)]
             + [c.QLR + c.KVLR]
             + [i * 128 for i in range(KT_Q)])
    tiles = []
    for m0 in order:
        t = np.zeros((c.HID, 128), np.float32)
        wsrc = wa[:, m0:m0 + 128]
        t[:, :wsrc.shape[1]] = wsrc
        tiles.append(t)
    wa_t = np.stack(tiles, axis=1)  # [HID, MT_A, 128]
    wa_t = wa_t.reshape(KT_HID, 128, MT_A, 128).transpose(1, 2, 0, 3)
    wa_t = np.ascontiguousarray(wa_t.reshape(128, MT_A * KT_HID * 128))
    wa_hi, wa_lo = split8(wa_t)
    return {
        "wa_h": wa_hi, "wa_l": wa_lo,
        "lnq": np.ascontiguousarray(
            np.asarray(q_ln_w).reshape(KT_Q, 128).T).astype(np.float32),
        "lnkv": np.ascontiguousarray(
            np.asarray(kv_ln_w).reshape(KT_KV, 128).T).astype(np.float32),
        "cosT": cosT,
        "sinT": sinT,
        "maskm": np.ascontiguousarray(maskm),
    }


def prep_group(c: Cfg, heads, w_qb, w_kvb, w_o, n_heads_total):
    wq = np.asarray(w_qb, np.float32).reshape(
        c.QLR, n_heads_total, c.DQK)[:, heads, :] * WS
    H_ = c.HPC
    # qp columns permuted per head-pair to [t, h2, 32] so rope runs on
    # 64-row halves; un-permuted by the qTs write AP in the kernel
    wq_pe = wq[:, :, c.DN:].reshape(c.QLR, H_ // 2, 2, 2, 32)
    wq_pe = wq_pe.transpose(0, 1, 3, 2, 4).reshape(c.QLR, -1)
    wq_g = np.concatenate(
        [wq[:, :, :c.DN].reshape(c.QLR, -1), wq_pe], axis=1)
    wkv = np.asarray(w_kvb, np.float32).reshape(
        c.KVLR, n_heads_total, c.DN + c.DV)[:, heads, :] * WS
    wkv_g = np.concatenate(
        [wkv[:, :, :c.DN].reshape(c.KVLR, -1),
         wkv[:, :, c.DN:].reshape(c.KVLR, -1)], axis=1)
    wo_g = np.asarray(w_o, np.float32).reshape(
        n_heads_total, c.DV, c.HID)[heads].reshape(-1, c.HID) * WS
    H = c.HPC
    KT_Q = c.QLR // 128
    KT_KV = c.KVLR // 128
    MT_QB = wq_g.shape[1] // 128
    MT_O = c.HID // 128
    wq_t = wq_g.reshape(KT_Q, 128, MT_QB, 128).transpose(1, 2, 0, 3)
    wq_t = np.ascontiguousarray(wq_t.reshape(128, MT_QB * KT_Q * 128))
    wkv_t = wkv_g.reshape(KT_KV, 128, wkv_g.shape[1]).transpose(1, 0, 2)
    wkv_t = np.ascontiguousarray(wkv_t.reshape(128, -1))
    wo_t = wo_g.reshape(H, 128, MT_O, 128).transpose(1, 2, 0, 3)
    wo_t = np.ascontiguousarray(wo_t.reshape(128, MT_O * H * 128))
    wq_h, wq_l = split8(wq_t)
    wkv_hi, wkv_lo = split8(wkv_t)
    wo_hi, wo_lo = split8(wo_t)
    MT_QB2 = MT_QB
    wq_hl = np.stack([wq_h.reshape(128, MT_QB2, KT_Q * 128),
                      wq_l.reshape(128, MT_QB2, KT_Q * 128)], axis=2)
    wo_hl = np.stack([wo_hi.reshape(128, MT_O, H * 128),
                      wo_lo.reshape(128, MT_O, H * 128)], axis=2)
    return {
        "wqb_hl": np.ascontiguousarray(wq_hl.reshape(128, -1)),
        "wkv_h": wkv_hi, "wkv_l": wkv_lo,
        "wo_hl": np.ascontiguousarray(wo_hl.reshape(128, -1)),
    }


_PROGRAM = None


def _get_program():
    global _PROGRAM
    if _PROGRAM is None:
        _PROGRAM = build_program(FULL)
    return _PROGRAM


def kernel(x, w_a, q_ln_w, kv_ln_w, w_qb, w_kvb, w_o):
    from concourse.bass_utils import run_bass_kernel_spmd

    c = FULL
    x = np.asarray(x, dtype=np.float32)
    B = x.shape[0]
    n_heads = w_qb.shape[1] // c.DQK
    n_groups = n_heads // c.HPC
    assert B * n_groups == c.NCORES and n_groups == c.GS

    nc = _get_program()
    shared = prep_shared(c, np.asarray(w_a), np.asarray(q_ln_w),
                         np.asarray(kv_ln_w))
    groups = [
        prep_group(c, slice(g * c.HPC, (g + 1) * c.HPC), np.asarray(w_qb),
                   np.asarray(w_kvb), np.asarray(w_o), n_heads)
        for g in range(n_groups)
    ]

    in_maps = []
    for core in range(c.NCORES):
        b, g = divmod(core, n_groups)
        sl = slice(g * c.SL, (g + 1) * c.SL)
        xtl = np.ascontiguousarray(x[b].T[:, sl]).reshape(
            c.HID // 128, 128, c.SL)
        xtl = np.ascontiguousarray(xtl.transpose(1, 0, 2).reshape(128, -1))
        xtl_h, xtl_l = split8(xtl)
        in_maps.append({
            "xh": xtl_h, "xl": xtl_l,
            "cosA": np.ascontiguousarray(shared["cosT"][:, sl]),
            "sinA": np.ascontiguousarray(shared["sinT"][:, sl]),
            **shared, **groups[g],
        })

    res = run_bass_kernel_spmd(nc, in_maps, core_ids=list(range(c.NCORES)))
    outs = [r["outT"] for r in res.results]
    result = np.empty((B, c.S, c.HID), dtype=np.float32)
    inv = 1.0 / (WS * OS)
    for b in range(B):
        acc = outs[b * n_groups].copy()
        for g in range(1, n_groups):
            acc += outs[b * n_groups + g]
        result[b] = acc.T * inv
    return result


# revision 3
# speedup vs baseline: 1.0010x; 1.0010x over previous
"""MLA forward kernel for Trainium2, 8 NeuronCores — fp8 comp3 edition.

Sharding: 8 cores = 2 (batch) x 4 (head-groups of 10 heads), as baseline.

vs baseline:
  - A/B/C/E matmuls use fp8e4 DoubleRow "comp3": x ~ xh+xl, w ~ wh+wl (both
    e4m3, weights pre-scaled x32 on host), psum = xh@wh + xl@wh + xh@wl.
    Each DoubleRow instruction contracts 2 k-tiles at 0.5 cycles/row.
  - scores/AV stay bf16 (accuracy + layout simplicity).
  - activations gathered across cores as fp8 hi/lo (kv+rope early, q in one
    combined hi+lo AllGather); phase C is a single comp3 pass writing q
    directly as fp8 into qTs8 ([qn,qn,qp,qp] per head so the DoubleRow rhs
    slots line up; qn written once via a broadcast DMA).
  - scores are fp8 DoubleRow too: k_nope hi/lo ride the two k-slots of one
    instruction, k_pe hi/lo in a second K=64 instruction (q plain fp8, "comp2"
    on the k side); exp applies 1/sqrt(dqk) via the Act scale and is batched
    2-wide ([128,1024] per Act instruction).
  - phase E o-proj m-tiles are woven into the next attention chunk's head
    loop so the PE stays busy while Act works through the exp wall; osb is
    fp8 hi/lo at x4 scale and the host divides the final sum by 128.
"""

import math
import sys
from dataclasses import dataclass

if "/opt/trn_rl_repo" not in sys.path:
    sys.path.insert(0, "/opt/trn_rl_repo")

import ml_dtypes
import numpy as np

BF16 = ml_dtypes.bfloat16
E4M3 = ml_dtypes.float8_e4m3
WS = 32.0          # host weight pre-scale (power of 2)
OS = 4.0           # osb scale (via 0.25 ones column)


@dataclass(frozen=True)
class Cfg:
    HID: int = 5120
    S: int = 2048
    QLR: int = 1536
    KVLR: int = 512
    DN: int = 128
    DR: int = 64
    DV: int = 128
    HPC: int = 10
    CHUNK: int = 512
    GS: int = 1
    NCORES: int = 8
    EPS: float = 1e-6
    THETA: float = 10000.0

    @property
    def DQK(self):
        return self.DN + self.DR

    @property
    def PEH(self):
        return self.DR // 2

    @property
    def SL(self):
        return self.S // self.GS


FULL = Cfg(GS=4)
DR_MODE = True  # DoubleRow fp8


def build_program(c: Cfg, stop_after: str = "E"):
    import contextlib

    import concourse.bass as bass  # noqa: F401
    import concourse.mybir as mybir
    import concourse.tile as tile
    from concourse import bacc
    from concourse.masks import make_identity

    dt = mybir.dt
    BF = dt.bfloat16
    F32 = dt.float32
    FP8 = dt.float8e4
    Alu = mybir.AluOpType
    Act = mybir.ActivationFunctionType
    DRow = mybir.MatmulPerfMode.DoubleRow

    KT_HID = c.HID // 128
    KT_Q = c.QLR // 128
    KT_KV = c.KVLR // 128
    NQC = c.S // c.CHUNK
    GS = c.GS
    SL = c.SL
    ST = c.S // 128
    H = c.HPC
    TPC = c.CHUNK // 128
    MT_QN = H * c.DN // 128
    MT_QP = H * c.DR // 128
    QROWS = H * (c.DN + c.DR)
    KROWS = H * c.DN
    VCOLS = H * c.DV
    MT_O = c.HID // 128
    MT_A = KT_Q + KT_KV + 1
    SCALE = 1.0 / math.sqrt(c.DQK)
    EV = SCALE / WS   # eviction scale for q tiles
    IV = 1.0 / WS     # eviction scale for a/kv tiles

    assert c.DN == 128 and c.DV == 128 and c.DR == 64 and H % 2 == 0
    assert KT_HID % 2 == 0 and KT_Q % 2 == 0 and KT_KV % 2 == 0
    _PH = ["A", "B", "C", "D", "E"]
    enabled = set(_PH[:_PH.index(stop_after) + 1])

    nc = bacc.Bacc("TRN2", num_devices=(c.NCORES if GS > 1 else None))
    xh = nc.dram_tensor("xh", [128, KT_HID * SL], FP8, kind="ExternalInput")
    xl = nc.dram_tensor("xl", [128, KT_HID * SL], FP8, kind="ExternalInput")
    wa_h = nc.dram_tensor("wa_h", [128, MT_A * KT_HID * 128], FP8,
                          kind="ExternalInput")
    wa_l = nc.dram_tensor("wa_l", [128, MT_A * KT_HID * 128], FP8,
                          kind="ExternalInput")
    wqb_hl = nc.dram_tensor(
        "wqb_hl", [128, (MT_QN + MT_QP) * 2 * KT_Q * 128], FP8,
        kind="ExternalInput")
    wkv_h = nc.dram_tensor("wkv_h", [128, KT_KV * (KROWS + VCOLS)], FP8,
                           kind="ExternalInput")
    wkv_l = nc.dram_tensor("wkv_l", [128, KT_KV * (KROWS + VCOLS)], FP8,
                           kind="ExternalInput")
    wo_hl = nc.dram_tensor("wo_hl", [128, MT_O * 2 * H * 128], FP8,
                           kind="ExternalInput")
    cosT = nc.dram_tensor("cosT", [128, c.S], BF, kind="ExternalInput")
    sinT = nc.dram_tensor("sinT", [128, c.S], BF, kind="ExternalInput")
    cosA = nc.dram_tensor("cosA", [128, SL], BF, kind="ExternalInput")
    sinA = nc.dram_tensor("sinA", [128, SL], BF, kind="ExternalInput")
    lnq = nc.dram_tensor("lnq", [128, KT_Q], F32, kind="ExternalInput")
    lnkv = nc.dram_tensor("lnkv", [128, KT_KV], F32, kind="ExternalInput")
    maskm = nc.dram_tensor("maskm", [128, TPC, c.CHUNK], BF,
                           kind="ExternalInput")
    outT = nc.dram_tensor("outT", [c.HID, c.S], F32, kind="ExternalOutput")
    qTs = nc.dram_tensor("qTs", [QROWS, c.S], BF, kind="Internal")
    qTs8 = nc.dram_tensor("qTs8", [H * 512, c.S], FP8, kind="Internal")
    # gather buffers: kv hi(4kt)+lo(4kt) fp8; kpe bf16; q hi / q lo fp8
    aglkv = nc.dram_tensor("aglkv", [2 * KT_KV * 128, SL], FP8,
                           kind="Internal")
    aglpe = nc.dram_tensor("aglpe", [c.DR, SL], BF, kind="Internal")
    aglq = nc.dram_tensor("aglq", [2 * KT_Q * 128, SL], FP8, kind="Internal")
    if GS > 1:
        aggkv = nc.dram_tensor("aggkv", [GS * 2 * KT_KV * 128, SL], FP8,
                               kind="Internal")
        aggpe = nc.dram_tensor("aggpe", [GS * c.DR, SL], BF, kind="Internal")
        aggq = nc.dram_tensor("aggq", [GS * 2 * KT_Q * 128, SL], FP8,
                              kind="Internal")
    else:
        aggkv, aggpe, aggq = aglkv, aglpe, aglq

    xh_r = xh.ap().rearrange("p (t s) -> p t s", s=SL)
    xl_r = xl.ap().rearrange("p (t s) -> p t s", s=SL)
    wa_h_r = wa_h.ap().rearrange("p (mt k m) -> p mt (k m)", mt=MT_A, m=128)
    wa_l_r = wa_l.ap().rearrange("p (mt k m) -> p mt (k m)", mt=MT_A, m=128)
    wqb_hl_r = wqb_hl.ap().rearrange("p (mt t k m) -> p mt (t k m)",
                                     mt=MT_QN + MT_QP, m=128, t=2)
    wkv_h_r = wkv_h.ap().rearrange("p (k m) -> p k m", k=KT_KV)
    wkv_l_r = wkv_l.ap().rearrange("p (k m) -> p k m", k=KT_KV)
    wo_hl_r = wo_hl.ap().rearrange("p (mt t k m) -> p mt (t k m)",
                                   mt=MT_O, m=128, t=2)
    aggkv_r = aggkv.ap().rearrange("(g t p) s -> g p t s", g=GS, p=128)
    aggpe_r = aggpe.ap().rearrange("(g p) s -> g p s", g=GS)
    aggq_r = aggq.ap().rearrange("(g t p) s -> g p t s", g=GS, p=128)
    aglkv_r = aglkv.ap().rearrange("(t p) s -> p t s", p=128)
    aglq_r = aglq.ap().rearrange("(t p) s -> p t s", p=128)
    qTs_ap = qTs.ap()
    qTs8_r = qTs8.ap().rearrange("(h q) s -> h q s", q=512)
    outT_ap = outT.ap()
    groups = [[b * GS + j for j in range(GS)] for b in range(c.NCORES // GS)]

    def dr_terms(ps, wh_t, wl_t, xh_t, xl_t, nk, msl, hi_only=False,
                 lo_only=False, start=True, stop=True):
        """Emit comp3 DoubleRow matmuls into psum `ps`.

        wh_t/wl_t: [128, nk, M] fp8 SBUF; xh_t/xl_t: [128, nk, N].
        msl: slice on the M dim of the weight tiles.
        """
        terms = []
        if not lo_only:
            terms += [(wh_t, xh_t), (wl_t, xh_t)]
        if not hi_only:
            terms += [(wh_t, xl_t)]
        np_ = nk // 2
        n_ins = len(terms) * np_
        i = 0
        for (wt, xt) in terms:
            for j in range(np_):
                nc.tensor.matmul(
                    ps, wt[:, 2 * j:2 * j + 2, msl],
                    xt[:, 2 * j:2 * j + 2, :],
                    start=(start and i == 0),
                    stop=(stop and i == n_ins - 1),
                    perf_mode=DRow)
                i += 1

    def emit_rope(pool, dst64, src64, cos_ap, sin_ap, W, p0=0, ph=None):
        ph = ph or c.PEH
        t1, t2 = src64[0:ph], src64[ph:2 * ph]
        d1, d2 = dst64[0:ph], dst64[ph:2 * ph]
        c1, s1 = cos_ap[p0:p0 + ph], sin_ap[p0:p0 + ph]
        c2, s2 = cos_ap[p0 + ph:p0 + 2 * ph], sin_ap[p0 + ph:p0 + 2 * ph]
        ra = pool.tile([ph, W], BF, tag="rope_a", name="rope_a")
        rb = pool.tile([ph, W], BF, tag="rope_b", name="rope_b")
        nc.vector.tensor_tensor(out=ra, in0=t1, in1=c1, op=Alu.mult)
        nc.vector.tensor_tensor(out=rb, in0=t2, in1=s2, op=Alu.mult)
        nc.vector.tensor_tensor(out=d1, in0=ra, in1=rb, op=Alu.subtract)
        nc.vector.tensor_tensor(out=ra, in0=t2, in1=c2, op=Alu.mult)
        nc.vector.tensor_tensor(out=rb, in0=t1, in1=s1, op=Alu.mult)
        nc.vector.tensor_tensor(out=d2, in0=ra, in1=rb, op=Alu.add)

    with tile.TileContext(nc, pool_alloc_mode="queue") as tc:
        with contextlib.ExitStack() as top:
            pers = top.enter_context(tc.tile_pool(name="pers", bufs=1))
            cos_sb = pers.tile([128, c.S], BF, tag="cos_sb")
            sin_sb = pers.tile([128, c.S], BF, tag="sin_sb")
            cosa_sb = pers.tile([128, SL], BF, tag="cosa_sb")
            sina_sb = pers.tile([128, SL], BF, tag="sina_sb")
            lnq_sb = pers.tile([128, KT_Q], F32, tag="lnq_sb")
            lnkv_sb = pers.tile([128, KT_KV], F32, tag="lnkv_sb")
            mask_sb = pers.tile([128, TPC, c.CHUNK], BF, tag="mask_sb")
            ident = pers.tile([128, 128], BF, tag="ident")
            ones_f = pers.tile([1, 128], F32, tag="ones_f")
            ones_c = pers.tile([128, 1], BF, tag="ones_c")
            eps_sb = pers.tile([1, 1], F32, tag="eps_sb")
            nc.vector.memset(eps_sb, c.EPS)
            kpe = pers.tile([c.DR, c.S], BF, tag="kpe")
            nc.sync.dma_start(out=cos_sb, in_=cosT.ap())
            nc.sync.dma_start(out=sin_sb, in_=sinT.ap())
            nc.sync.dma_start(out=cosa_sb, in_=cosA.ap())
            nc.sync.dma_start(out=sina_sb, in_=sinA.ap())
            nc.sync.dma_start(out=lnq_sb, in_=lnq.ap())
            nc.sync.dma_start(out=lnkv_sb, in_=lnkv.ap())
            nc.sync.dma_start(out=mask_sb, in_=maskm.ap())
            make_identity(nc, ident)
            nc.vector.memset(ones_f, 1.0)
            nc.vector.memset(ones_c, 1.0)

            # -------- phase A ------------------------------------------------
            with contextlib.ExitStack() as st:
                pax = st.enter_context(tc.tile_pool(name="pax", bufs=1))
                paw = st.enter_context(tc.tile_pool(name="paw", bufs=3))
                pat = st.enter_context(tc.tile_pool(name="pat", bufs=2))
                paa = st.enter_context(tc.tile_pool(name="paa", bufs=1))
                paps = st.enter_context(
                    tc.tile_pool(name="paps", bufs=3, space="PSUM"))
                pssq = st.enter_context(
                    tc.tile_pool(name="pssq", bufs=1, space="PSUM"))
                pbc = st.enter_context(
                    tc.tile_pool(name="pbc", bufs=2, space="PSUM"))

                xh_sb = pax.tile([128, KT_HID, SL], FP8, tag="xh_sb")
                xl_sb = pax.tile([128, KT_HID, SL], FP8, tag="xl_sb")
                nc.sync.dma_start(out=xh_sb, in_=xh_r)
                nc.sync.dma_start(out=xl_sb, in_=xl_r)
                aq_c = paa.tile([128, KT_Q, SL], BF, tag="aq_c")
                akv_c = paa.tile([128, KT_KV, SL], BF, tag="akv_c")
                aq8h = paa.tile([128, KT_Q, SL], FP8, tag="aq8h")
                aq8l = paa.tile([128, KT_Q, SL], FP8, tag="aq8l")
                akv8 = paa.tile([128, 2 * KT_KV, SL], FP8, tag="akv8")
                ssq_q = pssq.tile([1, SL], F32, tag="ssq_q")
                ssq_kv = pssq.tile([1, SL], F32, tag="ssq_kv")
                kperaw = pat.tile([c.DR, SL], BF, tag="kperaw", bufs=1)
                kpel = pat.tile([c.DR, SL], BF, tag="kpel", bufs=1)

                def normalize(ssq, ln_sb, ktn, denom, dst):
                    rn = pat.tile([1, c.CHUNK], F32, tag="rn", name="rn")
                    nc.scalar.activation(
                        rn, ssq, Act.Sqrt, bias=eps_sb, scale=1.0 / denom)
                    rnr = pat.tile([1, c.CHUNK], F32, tag="rnr", name="rnr")
                    nc.vector.reciprocal(rnr, rn)
                    bc = pbc.tile([128, c.CHUNK], F32, tag="bc", name="bc")
                    nc.tensor.matmul(bc, ones_f, rnr, start=True, stop=True)
                    for t in range(ktn):
                        tgt = dst[:, t, :]
                        nc.vector.scalar_tensor_tensor(
                            out=tgt, in0=tgt, scalar=ln_sb[:, t:t + 1],
                            in1=bc, op0=Alu.mult, op1=Alu.mult)

                mtiles = ([("kv", i) for i in range(KT_KV)]
                          + [("pe", 0)]
                          + [("q", i) for i in range(KT_Q)])
                for mti, (seg, ti) in enumerate(mtiles):
                    mw = c.DR if seg == "pe" else 128
                    wah_sb = paw.tile([128, KT_HID, 128], FP8, tag="wah",
                                      name="wah")
                    wal_sb = paw.tile([128, KT_HID, 128], FP8, tag="wal",
                                      name="wal")
                    nc.sync.dma_start(
                        out=wah_sb,
                        in_=wa_h_r[:, mti].rearrange("p (k m) -> p k m",
                                                     m=128))
                    nc.sync.dma_start(
                        out=wal_sb,
                        in_=wa_l_r[:, mti].rearrange("p (k m) -> p k m",
                                                     m=128))
                    ps = paps.tile([128, c.CHUNK], F32, tag="aps", name="ps")
                    dr_terms(ps[:mw], wah_sb, wal_sb, xh_sb, xl_sb,
                             KT_HID, slice(0, mw))
                    if seg == "q":
                        nc.scalar.activation(aq_c[:, ti, :], ps, Act.Copy,
                                             scale=IV)
                        sq = pat.tile([128, c.CHUNK], BF, tag="sq", bufs=3,
                                      name="sq")
                        nc.scalar.activation(sq, ps, Act.Square, scale=IV)
                        nc.tensor.matmul(
                            ssq_q, ones_c, sq,
                            start=(ti == 0), stop=(ti == KT_Q - 1))
                    elif seg == "kv":
                        nc.scalar.activation(akv_c[:, ti, :], ps, Act.Copy,
                                             scale=IV)
                        sq = pat.tile([128, c.CHUNK], BF, tag="sq", bufs=3,
                                      name="sq")
                        nc.scalar.activation(sq, ps, Act.Square, scale=IV)
                        nc.tensor.matmul(
                            ssq_kv, ones_c, sq,
                            start=(ti == 0), stop=(ti == KT_KV - 1))
                    else:
                        nc.scalar.activation(kperaw, ps[:mw], Act.Copy,
                                             scale=IV)
                    if seg == "pe":
                        # kv part complete: normalize, rope, split, gather
                        normalize(ssq_kv, lnkv_sb, KT_KV, c.KVLR, akv_c)
                        emit_rope(pat, kpel, kperaw, cosa_sb, sina_sb,
                                  c.CHUNK)
                        for t in range(KT_KV):
                            nc.scalar.copy(akv8[:, t, :], akv_c[:, t, :])
                            nc.vector.scalar_tensor_tensor(
                                out=akv8[:, KT_KV + t, :],
                                in0=akv_c[:, t, :], scalar=1.0,
                                in1=akv8[:, t, :],
                                op0=Alu.mult, op1=Alu.subtract)
                        nc.sync.dma_start(out=aglkv_r, in_=akv8)
                        nc.sync.dma_start(out=aglpe.ap(), in_=kpel)
                        if GS > 1:
                            nc.gpsimd.collective_compute(
                                "AllGather", Alu.bypass,
                                replica_groups=groups,
                                ins=[aglkv.ap()], outs=[aggkv.ap()])
                            nc.gpsimd.collective_compute(
                                "AllGather", Alu.bypass,
                                replica_groups=groups,
                                ins=[aglpe.ap()], outs=[aggpe.ap()])
                normalize(ssq_q, lnq_sb, KT_Q, c.QLR, aq_c)
                for t in range(KT_Q):
                    nc.scalar.copy(aq8h[:, t, :], aq_c[:, t, :])
                    nc.vector.scalar_tensor_tensor(
                        out=aq8l[:, t, :], in0=aq_c[:, t, :], scalar=1.0,
                        in1=aq8h[:, t, :], op0=Alu.mult, op1=Alu.subtract)
                nc.sync.dma_start(out=aglq_r[:, 0:KT_Q, :], in_=aq8h)
                nc.sync.dma_start(out=aglq_r[:, KT_Q:2 * KT_Q, :], in_=aq8l)
                if GS > 1:
                    nc.gpsimd.collective_compute(
                        "AllGather", Alu.bypass, replica_groups=groups,
                        ins=[aglq.ap()], outs=[aggq.ap()])

            for g in range(GS):
                nc.sync.dma_start(out=kpe[:, g * SL:(g + 1) * SL],
                                  in_=aggpe_r[g])

            # -------- phase B: kv up-projection ------------------------------
            if "B" in enabled:
                pkv = top.enter_context(tc.tile_pool(name="pkv", bufs=1))
                kn = pkv.tile([128, H, 2, c.S], FP8, tag="kn")
                vv = pkv.tile([128, ST, H, c.DV + 1], BF, tag="vv")
                nc.vector.memset(vv[:, :, :, c.DV:], 1.0 / OS)
                kpe8 = pkv.tile([c.DR, 2, c.S], FP8, tag="kpe8")
                nc.scalar.copy(kpe8[:, 0, :], kpe)
                nc.vector.scalar_tensor_tensor(
                    out=kpe8[:, 1, :], in0=kpe, scalar=1.0,
                    in1=kpe8[:, 0, :], op0=Alu.mult, op1=Alu.subtract)

                with contextlib.ExitStack() as st:
                    pbw = st.enter_context(tc.tile_pool(name="pbw", bufs=1))
                    pbps = st.enter_context(
                        tc.tile_pool(name="pbps", bufs=3, space="PSUM"))
                    wkvh_sb = pbw.tile([128, KT_KV, KROWS + VCOLS], FP8,
                                       tag="wkvh")
                    wkvl_sb = pbw.tile([128, KT_KV, KROWS + VCOLS], FP8,
                                       tag="wkvl")
                    nc.sync.dma_start(out=wkvh_sb, in_=wkv_h_r)
                    nc.sync.dma_start(out=wkvl_sb, in_=wkv_l_r)
                    akvfh = pbw.tile([128, KT_KV, c.S], FP8, tag="akvfh")
                    akvfl = pbw.tile([128, KT_KV, c.S], FP8, tag="akvfl")
                    for g in range(GS):
                        sl_ = slice(g * SL, (g + 1) * SL)
                        nc.sync.dma_start(out=akvfh[:, :, sl_],
                                          in_=aggkv_r[g, :, 0:KT_KV, :])
                        nc.sync.dma_start(out=akvfl[:, :, sl_],
                                          in_=aggkv_r[g, :, KT_KV:2 * KT_KV, :])
                    ev_flip = [0]

                    def evict(dst, src, scale):
                        # alternate Act / DVE to balance engine load
                        if ev_flip[0] % 2 == 0:
                            nc.scalar.activation(dst, src, Act.Copy,
                                                 scale=scale)
                        else:
                            nc.vector.tensor_scalar_mul(dst, src, scale)
                        ev_flip[0] += 1

                    for mt in range(H):
                        msl = slice(mt * 128, (mt + 1) * 128)
                        for qc in range(NQC):
                            cs = slice(qc * c.CHUNK, (qc + 1) * c.CHUNK)
                            ps = pbps.tile([128, c.CHUNK], F32, tag="kps")
                            dr_terms(ps, wkvh_sb, wkvl_sb,
                                     akvfh[:, :, cs], akvfl[:, :, cs],
                                     KT_KV, msl)
                            nc.scalar.activation(kn[:, mt, 0, cs], ps,
                                                 Act.Copy, scale=IV)
                            nc.vector.scalar_tensor_tensor(
                                out=kn[:, mt, 1, cs], in0=ps, scalar=IV,
                                in1=kn[:, mt, 0, cs],
                                op0=Alu.mult, op1=Alu.subtract)
                    vch = []
                    v0 = 0
                    while v0 < VCOLS:
                        vw = min(512, VCOLS - v0)
                        vch.append((v0, vw))
                        v0 += vw
                    for stt_ in range(ST):
                        ss = slice(stt_ * 128, (stt_ + 1) * 128)
                        for v0, vw in vch:
                            vsl = slice(KROWS + v0, KROWS + v0 + vw)
                            ps = pbps.tile([128, 512], F32, tag="vps")
                            dr_terms(ps[:, :vw], akvfh[:, :, ss],
                                     akvfl[:, :, ss], wkvh_sb[:, :, vsl],
                                     wkvl_sb[:, :, vsl], KT_KV,
                                     slice(0, 128))
                            h0, hn = v0 // c.DV, vw // c.DV
                            evict(
                                vv[:, stt_, h0:h0 + hn, 0:c.DV],
                                ps[:, :vw].rearrange("p (h d) -> p h d",
                                                     d=c.DV), IV)

            # -------- phase C: q up-projection (single comp3 pass) -----------
            if "C" in enabled:
                with contextlib.ExitStack() as st:
                    pcq = st.enter_context(tc.tile_pool(name="pcq", bufs=1))
                    pcw = st.enter_context(tc.tile_pool(name="pcw", bufs=3))
                    pce = st.enter_context(tc.tile_pool(name="pce", bufs=3))
                    pcps = st.enter_context(
                        tc.tile_pool(name="pcps", bufs=4, space="PSUM"))
                    aqfh = pcq.tile([128, KT_Q, c.S], FP8, tag="aqfh")
                    aqfl = pcq.tile([128, KT_Q, c.S], FP8, tag="aqfl")
                    for g in range(GS):
                        sl_ = slice(g * SL, (g + 1) * SL)
                        nc.sync.dma_start(out=aqfh[:, :, sl_],
                                          in_=aggq_r[g, :, 0:KT_Q, :])
                        nc.sync.dma_start(out=aqfl[:, :, sl_],
                                          in_=aggq_r[g, :, KT_Q:2 * KT_Q, :])
                    mt_order = []
                    for j in range(MT_QP):
                        mt_order += [2 * j, 2 * j + 1, MT_QN + j]
                    mt_order += list(range(2 * MT_QP, MT_QN))
                    for mt in mt_order:
                        wq_sb = pcw.tile([128, 2, KT_Q, 128], FP8, tag="wq")
                        nc.sync.dma_start(
                            out=wq_sb,
                            in_=wqb_hl_r[:, mt].rearrange(
                                "p (t k m) -> p t k m", t=2, m=128))
                        for qc in range(NQC):
                            col = qc * c.CHUNK
                            cs = slice(col, col + c.CHUNK)
                            ps = pcps.tile([128, c.CHUNK], F32, tag="qps")
                            dr_terms(ps, wq_sb[:, 0], wq_sb[:, 1],
                                     aqfh[:, :, cs], aqfl[:, :, cs],
                                     KT_Q, slice(0, 128))
                            if mt >= MT_QN:
                                qsb = pce.tile([128, c.CHUNK], BF, tag="qsb")
                                nc.scalar.activation(qsb, ps, Act.Copy,
                                                     scale=IV)
                                roped = pce.tile([128, c.CHUNK], FP8,
                                                 tag="roped")
                                emit_rope(pce, roped, qsb,
                                          cos_sb[:, cs], sin_sb[:, cs],
                                          c.CHUNK, ph=64)
                                j2 = mt - MT_QN
                                for sl8 in (256, 384):
                                    for j in (0, 1):
                                        dst = qTs8_r[
                                            2 * j2:2 * j2 + 2,
                                            sl8 + j * 32:sl8 + j * 32 + 32,
                                            cs]
                                        nc.sync.dma_start(
                                            out=dst,
                                            in_=roped[j * 64:(j + 1) * 64])
                            else:
                                qf8 = pce.tile([128, c.CHUNK], FP8,
                                               tag="qf8")
                                nc.scalar.activation(qf8, ps, Act.Copy,
                                                     scale=IV)
                                dst = qTs8_r[mt, 0:256, cs].rearrange(
                                    "(two p) s -> p two s", p=128)
                                nc.sync.dma_start(
                                    out=dst,
                                    in_=qf8[:, None, :].broadcast_to(
                                        [128, 2, c.CHUNK]))
            # -------- phase D: attention -------------------------------------
            if "D" in enabled:
                pot = top.enter_context(tc.tile_pool(name="pot", bufs=1))
                oT_h = pot.tile([128, H, c.S], FP8, tag="oT_h")
                oT_l = pot.tile([128, H, c.S], FP8, tag="oT_l")

                with contextlib.ExitStack() as st:
                    pdp = st.enter_context(tc.tile_pool(name="pdp", bufs=2))
                    pdq = st.enter_context(tc.tile_pool(name="pdq", bufs=3))
                    pde = st.enter_context(tc.tile_pool(name="pde", bufs=6))
                    pew = st.enter_context(tc.tile_pool(name="pew", bufs=3))
                    peo = st.enter_context(tc.tile_pool(name="peo", bufs=3))
                    pds = st.enter_context(
                        tc.tile_pool(name="pds", bufs=2, space="PSUM"))
                    pdo = st.enter_context(
                        tc.tile_pool(name="pdo", bufs=1, space="PSUM"))
                    pdt = st.enter_context(
                        tc.tile_pool(name="pdt", bufs=1, space="PSUM"))
                    peps = st.enter_context(
                        tc.tile_pool(name="peps", bufs=2, space="PSUM"))

                    def e_tiles(qc, mts):
                        cs = slice(qc * c.CHUNK, (qc + 1) * c.CHUNK)
                        for mt in mts:
                            wo_sb = pew.tile([128, 2, H, 128], FP8, tag="wo")
                            nc.sync.dma_start(
                                out=wo_sb,
                                in_=wo_hl_r[:, mt].rearrange(
                                    "p (t k m) -> p t k m", t=2, m=128))
                            ps = peps.tile([128, c.CHUNK], F32, tag="ops")
                            dr_terms(ps, wo_sb[:, 0], wo_sb[:, 1],
                                     oT_h[:, :, cs],
                                     oT_l[:, :, cs], H, slice(0, 128))
                            ob = peo.tile([128, c.CHUNK], F32, tag="ob")
                            if mt % 2 == 0:
                                nc.scalar.copy(ob, ps)
                            else:
                                nc.vector.tensor_copy(out=ob, in_=ps)
                            nc.sync.dma_start(
                                out=outT_ap[mt * 128:(mt + 1) * 128, cs],
                                in_=ob)

                    def d_chunk(qc, eq):
                        col = qc * c.CHUNK
                        kmax = min(TPC * qc + TPC, ST)
                        for h in range(H):
                            qf = pdq.tile([128, 4, c.CHUNK], FP8, tag="qf")
                            nc.sync.dma_start(
                                out=qf,
                                in_=qTs8_r[h, :, col:col + c.CHUNK].rearrange(
                                    "(fo p) s -> p fo s", p=128))
                            probs = pdp.tile([128, ST, c.CHUNK], BF,
                                             tag="probs")
                            for k2 in range(kmax // 2):
                                ps = pds.tile([128, 2, c.CHUNK], F32,
                                              tag="sc")
                                for i in (0, 1):
                                    kt = 2 * k2 + i
                                    ksl = slice(kt * 128, (kt + 1) * 128)
                                    nc.tensor.matmul(
                                        ps[:, i, :], kn[:, h, :, ksl],
                                        qf[:, 0:2, :], start=True,
                                        stop=False, perf_mode=DRow)
                                    nc.tensor.matmul(
                                        ps[:, i, :], kpe8[:, :, ksl],
                                        qf[0:64, 2:4, :], start=False,
                                        stop=True, perf_mode=DRow)
                                nc.scalar.activation(
                                    probs[:, 2 * k2:2 * k2 + 2, :], ps,
                                    Act.Exp, scale=SCALE)
                                for i in (0, 1):
                                    kt = 2 * k2 + i
                                    d = kt - TPC * qc
                                    if d >= 0:
                                        nc.vector.tensor_tensor(
                                            out=probs[:, kt, :],
                                            in0=probs[:, kt, :],
                                            in1=mask_sb[:, d, :],
                                            op=Alu.mult)
                            for q2 in range(TPC):
                                qt = TPC * qc + q2
                                po = pdo.tile([128, c.DV + 1], F32, tag="po")
                                for kt in range(qt + 1):
                                    nc.tensor.matmul(
                                        po,
                                        probs[:, kt, q2 * 128:(q2 + 1) * 128],
                                        vv[:, kt, h, :],
                                        start=(kt == 0), stop=(kt == qt))
                                rec = pde.tile([128, 1], F32, tag="rec")
                                nc.vector.reciprocal(
                                    rec, po[:, c.DV:c.DV + 1])
                                osb = pde.tile([128, c.DV], BF, tag="osb")
                                nc.vector.tensor_scalar_mul(
                                    osb, po[:, :c.DV], rec)
                                pt = pdt.tile([128, 128], BF, tag="pt")
                                nc.tensor.transpose(pt, osb, ident)
                                ql_ = slice(qt * 128, (qt + 1) * 128)
                                nc.vector.tensor_copy(
                                    out=oT_h[:, h, ql_], in_=pt)
                                nc.vector.scalar_tensor_tensor(
                                    out=oT_l[:, h, ql_], in0=pt, scalar=1.0,
                                    in1=oT_h[:, h, ql_],
                                    op0=Alu.mult, op1=Alu.subtract)
                            if eq is not None and "E" in enabled:
                                e_tiles(eq, range(4 * h, 4 * h + 4))

                    d_chunk(0, None)
                    d_chunk(1, 0)
                    d_chunk(2, 1)
                    d_chunk(3, 2)
                    if "E" in enabled:
                        e_tiles(3, range(MT_O))

    nc.compile()
    return nc


# ---------------------------------------------------------------------------
# host-side input preparation
# ---------------------------------------------------------------------------

def split8(x):
    hi = np.asarray(x, np.float32).astype(E4M3)
    lo = (np.asarray(x, np.float32) - hi.astype(np.float32)).astype(E4M3)
    return hi, lo


def prep_shared(c: Cfg, w_a, q_ln_w, kv_ln_w):
    KT_Q = c.QLR // 128
    KT_KV = c.KVLR // 128
    TPC = c.CHUNK // 128
    half = c.PEH
    inv_freq = 1.0 / (c.THETA ** (np.arange(half, dtype=np.float32) / half))
    ang = np.arange(c.S, dtype=np.float32)[:, None] * inv_freq[None, :]
    cosT = np.ascontiguousarray(
        np.tile(np.cos(ang).T, (128 // half, 1))).astype(BF16)
    sinT = np.ascontiguousarray(
        np.tile(np.sin(ang).T, (128 // half, 1))).astype(BF16)
    k_idx = np.arange(128)[:, None]
    q_idx = np.arange(c.CHUNK)[None, :]
    maskm = np.stack(
        [(k_idx <= q_idx - 128 * d) for d in range(TPC)], axis=1
    ).astype(BF16)
    MT_A = KT_Q + KT_KV + 1
    KT_HID = c.HID // 128
    wa = np.asarray(w_a, np.float32) * WS
    order = ([c.QLR + i * 128 for i in range(KT_KV)]
             + [c.QLR + c.KVLR]
             + [i * 128 for i in range(KT_Q)])
    tiles = []
    for m0 in order:
        t = np.zeros((c.HID, 128), np.float32)
        wsrc = wa[:, m0:m0 + 128]
        t[:, :wsrc.shape[1]] = wsrc
        tiles.append(t)
    wa_t = np.stack(tiles, axis=1)  # [HID, MT_A, 128]
    wa_t = wa_t.reshape(KT_HID, 128, MT_A, 128).transpose(1, 2, 0, 3)
    wa_t = np.ascontiguousarray(wa_t.reshape(128, MT_A * KT_HID * 128))
    wa_hi, wa_lo = split8(wa_t)
    return {
        "wa_h": wa_hi, "wa_l": wa_lo,
        "lnq": np.ascontiguousarray(
            np.asarray(q_ln_w).reshape(KT_Q, 128).T).astype(np.float32),
        "lnkv": np.ascontiguousarray(
            np.asarray(kv_ln_w).reshape(KT_KV, 128).T).astype(np.float32),
        "cosT": cosT,
        "sinT": sinT,
        "maskm": np.ascontiguousarray(maskm),
    }


def prep_group(c: Cfg, heads, w_qb, w_kvb, w_o, n_heads_total):
    wq = np.asarray(w_qb, np.float32).reshape(
        c.QLR, n_heads_total, c.DQK)[:, heads, :] * WS
    H_ = c.HPC
    # qp columns permuted per head-pair to [t, h2, 32] so rope runs on
    # 64-row halves; un-permuted by the qTs write AP in the kernel
    wq_pe = wq[:, :, c.DN:].reshape(c.QLR, H_ // 2, 2, 2, 32)
    wq_pe = wq_pe.transpose(0, 1, 3, 2, 4).reshape(c.QLR, -1)
    wq_g = np.concatenate(
        [wq[:, :, :c.DN].reshape(c.QLR, -1), wq_pe], axis=1)
    wkv = np.asarray(w_kvb, np.float32).reshape(
        c.KVLR, n_heads_total, c.DN + c.DV)[:, heads, :] * WS
    wkv_g = np.concatenate(
        [wkv[:, :, :c.DN].reshape(c.KVLR, -1),
         wkv[:, :, c.DN:].reshape(c.KVLR, -1)], axis=1)
    wo_g = np.asarray(w_o, np.float32).reshape(
        n_heads_total, c.DV, c.HID)[heads].reshape(-1, c.HID) * WS
    H = c.HPC
    KT_Q = c.QLR // 128
    KT_KV = c.KVLR // 128
    MT_QB = wq_g.shape[1] // 128
    MT_O = c.HID // 128
    wq_t = wq_g.reshape(KT_Q, 128, MT_QB, 128).transpose(1, 2, 0, 3)
    wq_t = np.ascontiguousarray(wq_t.reshape(128, MT_QB * KT_Q * 128))
    wkv_t = wkv_g.reshape(KT_KV, 128, wkv_g.shape[1]).transpose(1, 0, 2)
    wkv_t = np.ascontiguousarray(wkv_t.reshape(128, -1))
    wo_t = wo_g.reshape(H, 128, MT_O, 128).transpose(1, 2, 0, 3)
    wo_t = np.ascontiguousarray(wo_t.reshape(128, MT_O * H * 128))
    wq_h, wq_l = split8(wq_t)
    wkv_hi, wkv_lo = split8(wkv_t)
    wo_hi, wo_lo = split8(wo_t)
    MT_QB2 = MT_QB
    wq_hl = np.stack([wq_h.reshape(128, MT_QB2, KT_Q * 128),
                      wq_l.reshape(128, MT_QB2, KT_Q * 128)], axis=2)
    wo_hl = np.stack([wo_hi.reshape(128, MT_O, H * 128),
                      wo_lo.reshape(128, MT_O, H * 128)], axis=2)
    return {
        "wqb_hl": np.ascontiguousarray(wq_hl.reshape(128, -1)),
        "wkv_h": wkv_hi, "wkv_l": wkv_lo,
        "wo_hl": np.ascontiguousarray(wo_hl.reshape(128, -1)),
    }


_PROGRAM = None


def _get_program():
    global _PROGRAM
    if _PROGRAM is None:
        _PROGRAM = build_program(FULL)
    return _PROGRAM


def kernel(x, w_a, q_ln_w, kv_ln_w, w_qb, w_kvb, w_o):
    from concourse.bass_utils import run_bass_kernel_spmd

    c = FULL
    x = np.asarray(x, dtype=np.float32)
    B = x.shape[0]
    n_heads = w_qb.shape[1] // c.DQK
    n_groups = n_heads // c.HPC
    assert B * n_groups == c.NCORES and n_groups == c.GS

    nc = _get_program()
    shared = prep_shared(c, np.asarray(w_a), np.asarray(q_ln_w),
                         np.asarray(kv_ln_w))
    groups = [
        prep_group(c, slice(g * c.HPC, (g + 1) * c.HPC), np.asarray(w_qb),
                   np.asarray(w_kvb), np.asarray(w_o), n_heads)
        for g in range(n_groups)
    ]

    in_maps = []
    for core in range(c.NCORES):
        b, g = divmod(core, n_groups)
        sl = slice(g * c.SL, (g + 1) * c.SL)
        xtl = np.ascontiguousarray(x[b].T[:, sl]).reshape(
            c.HID // 128, 128, c.SL)
        xtl = np.ascontiguousarray(xtl.transpose(1, 0, 2).reshape(128, -1))
        xtl_h, xtl_l = split8(xtl)
        in_maps.append({
            "xh": xtl_h, "xl": xtl_l,
            "cosA": np.ascontiguousarray(shared["cosT"][:, sl]),
            "sinA": np.ascontiguousarray(shared["sinT"][:, sl]),
            **shared, **groups[g],
        })

    res = run_bass_kernel_spmd(nc, in_maps, core_ids=list(range(c.NCORES)))
    outs = [r["outT"] for r in res.results]
    result = np.empty((B, c.S, c.HID), dtype=np.float32)
    inv = 1.0 / (WS * OS)
    for b in range(B):
        acc = outs[b * n_groups].copy()
        for g in range(1, n_groups):
            acc += outs[b * n_groups + g]
        result[b] = acc.T * inv
    return result


# revision 4
# speedup vs baseline: 1.0030x; 1.0020x over previous
"""MLA forward kernel for Trainium2, 8 NeuronCores — fp8 comp3 edition.

Sharding: 8 cores = 2 (batch) x 4 (head-groups of 10 heads), as baseline.

vs baseline:
  - A/B/C/E matmuls use fp8e4 DoubleRow "comp3": x ~ xh+xl, w ~ wh+wl (both
    e4m3, weights pre-scaled x32 on host), psum = xh@wh + xl@wh + xh@wl.
    Each DoubleRow instruction contracts 2 k-tiles at 0.5 cycles/row.
  - scores/AV stay bf16 (accuracy + layout simplicity).
  - activations gathered across cores as fp8 hi/lo; q gather split into a
    hi gather and a lo gather, with phase C split into a hi-pass (2 terms,
    partial qT in DRAM) and a lo-pass (1 term + merge) to shrink the
    collective exposure window.
  - attention out osb stored as fp8 hi/lo at x4 scale; o-proj is comp3;
    host divides the final sum by 128 (= 4 * 32).
  - exp batched 2-wide over score k-tiles ([128,1024] per Act instruction).
"""

import math
import sys
from dataclasses import dataclass

if "/opt/trn_rl_repo" not in sys.path:
    sys.path.insert(0, "/opt/trn_rl_repo")

import ml_dtypes
import numpy as np

BF16 = ml_dtypes.bfloat16
E4M3 = ml_dtypes.float8_e4m3
WS = 32.0          # host weight pre-scale (power of 2)
OS = 4.0           # osb scale (via 0.25 ones column)


@dataclass(frozen=True)
class Cfg:
    HID: int = 5120
    S: int = 2048
    QLR: int = 1536
    KVLR: int = 512
    DN: int = 128
    DR: int = 64
    DV: int = 128
    HPC: int = 10
    CHUNK: int = 512
    GS: int = 1
    NCORES: int = 8
    EPS: float = 1e-6
    THETA: float = 10000.0

    @property
    def DQK(self):
        return self.DN + self.DR

    @property
    def PEH(self):
        return self.DR // 2

    @property
    def SL(self):
        return self.S // self.GS


FULL = Cfg(GS=4)
DR_MODE = True  # DoubleRow fp8


def build_program(c: Cfg, stop_after: str = "E"):
    import contextlib

    import concourse.bass as bass  # noqa: F401
    import concourse.mybir as mybir
    import concourse.tile as tile
    from concourse import bacc
    from concourse.masks import make_identity

    dt = mybir.dt
    BF = dt.bfloat16
    F32 = dt.float32
    FP8 = dt.float8e4
    Alu = mybir.AluOpType
    Act = mybir.ActivationFunctionType
    DRow = mybir.MatmulPerfMode.DoubleRow

    KT_HID = c.HID // 128
    KT_Q = c.QLR // 128
    KT_KV = c.KVLR // 128
    NQC = c.S // c.CHUNK
    GS = c.GS
    SL = c.SL
    ST = c.S // 128
    H = c.HPC
    TPC = c.CHUNK // 128
    MT_QN = H * c.DN // 128
    MT_QP = H * c.DR // 128
    QROWS = H * (c.DN + c.DR)
    KROWS = H * c.DN
    VCOLS = H * c.DV
    MT_O = c.HID // 128
    MT_A = KT_Q + KT_KV + 1
    SCALE = 1.0 / math.sqrt(c.DQK)
    EV = SCALE / WS   # eviction scale for q tiles
    IV = 1.0 / WS     # eviction scale for a/kv tiles

    assert c.DN == 128 and c.DV == 128 and c.DR == 64 and H % 2 == 0
    assert KT_HID % 2 == 0 and KT_Q % 2 == 0 and KT_KV % 2 == 0
    _PH = ["A", "B", "C", "D", "E"]
    enabled = set(_PH[:_PH.index(stop_after) + 1])

    nc = bacc.Bacc("TRN2", num_devices=(c.NCORES if GS > 1 else None))
    xh = nc.dram_tensor("xh", [128, KT_HID * SL], FP8, kind="ExternalInput")
    xl = nc.dram_tensor("xl", [128, KT_HID * SL], FP8, kind="ExternalInput")
    wa_h = nc.dram_tensor("wa_h", [128, MT_A * KT_HID * 128], FP8,
                          kind="ExternalInput")
    wa_l = nc.dram_tensor("wa_l", [128, MT_A * KT_HID * 128], FP8,
                          kind="ExternalInput")
    wqb_hl = nc.dram_tensor(
        "wqb_hl", [128, (MT_QN + MT_QP) * 2 * KT_Q * 128], FP8,
        kind="ExternalInput")
    wkv_h = nc.dram_tensor("wkv_h", [128, KT_KV * (KROWS + VCOLS)], FP8,
                           kind="ExternalInput")
    wkv_l = nc.dram_tensor("wkv_l", [128, KT_KV * (KROWS + VCOLS)], FP8,
                           kind="ExternalInput")
    wo_hl = nc.dram_tensor("wo_hl", [128, MT_O * 2 * H * 128], FP8,
                           kind="ExternalInput")
    cosT = nc.dram_tensor("cosT", [128, c.S], BF, kind="ExternalInput")
    sinT = nc.dram_tensor("sinT", [128, c.S], BF, kind="ExternalInput")
    cosA = nc.dram_tensor("cosA", [128, SL], BF, kind="ExternalInput")
    sinA = nc.dram_tensor("sinA", [128, SL], BF, kind="ExternalInput")
    lnq = nc.dram_tensor("lnq", [128, KT_Q], F32, kind="ExternalInput")
    lnkv = nc.dram_tensor("lnkv", [128, KT_KV], F32, kind="ExternalInput")
    maskm = nc.dram_tensor("maskm", [128, TPC, c.CHUNK], BF,
                           kind="ExternalInput")
    outT = nc.dram_tensor("outT", [c.HID, c.S], F32, kind="ExternalOutput")
    qTs = nc.dram_tensor("qTs", [QROWS, c.S], BF, kind="Internal")
    qTs8 = nc.dram_tensor("qTs8", [H * 512, c.S], FP8, kind="Internal")
    # gather buffers: kv hi(4kt)+lo(4kt) fp8; kpe bf16; q hi / q lo fp8
    aglkv = nc.dram_tensor("aglkv", [(2 * KT_KV + 2) * 128, SL], FP8,
                           kind="Internal")
    aglq = nc.dram_tensor("aglq", [2 * KT_Q * 128, SL], FP8, kind="Internal")
    if GS > 1:
        aggkv = nc.dram_tensor("aggkv", [GS * (2 * KT_KV + 2) * 128, SL],
                               FP8, kind="Internal")
        aggq = nc.dram_tensor("aggq", [GS * 2 * KT_Q * 128, SL], FP8,
                              kind="Internal")
    else:
        aggkv, aggq = aglkv, aglq

    xh_r = xh.ap().rearrange("p (t s) -> p t s", s=SL)
    xl_r = xl.ap().rearrange("p (t s) -> p t s", s=SL)
    wa_h_r = wa_h.ap().rearrange("p (mt k m) -> p mt (k m)", mt=MT_A, m=128)
    wa_l_r = wa_l.ap().rearrange("p (mt k m) -> p mt (k m)", mt=MT_A, m=128)
    wqb_hl_r = wqb_hl.ap().rearrange("p (mt t k m) -> p mt (t k m)",
                                     mt=MT_QN + MT_QP, m=128, t=2)
    wkv_h_r = wkv_h.ap().rearrange("p (k m) -> p k m", k=KT_KV)
    wkv_l_r = wkv_l.ap().rearrange("p (k m) -> p k m", k=KT_KV)
    wo_hl_r = wo_hl.ap().rearrange("p (mt t k m) -> p mt (t k m)",
                                   mt=MT_O, m=128, t=2)
    aggkv_r = aggkv.ap().rearrange("(g t p) s -> g p t s", g=GS, p=128)
    aggq_r = aggq.ap().rearrange("(g t p) s -> g p t s", g=GS, p=128)
    aglkv_r = aglkv.ap().rearrange("(t p) s -> p t s", p=128)
    aglq_r = aglq.ap().rearrange("(t p) s -> p t s", p=128)
    qTs_ap = qTs.ap()
    qTs8_r = qTs8.ap().rearrange("(h q) s -> h q s", q=512)
    outT_ap = outT.ap()
    groups = [[b * GS + j for j in range(GS)] for b in range(c.NCORES // GS)]

    def dr_terms(ps, wh_t, wl_t, xh_t, xl_t, nk, msl, hi_only=False,
                 lo_only=False, start=True, stop=True):
        """Emit comp3 DoubleRow matmuls into psum `ps`.

        wh_t/wl_t: [128, nk, M] fp8 SBUF; xh_t/xl_t: [128, nk, N].
        msl: slice on the M dim of the weight tiles.
        """
        terms = []
        if not lo_only:
            terms += [(wh_t, xh_t), (wl_t, xh_t)]
        if not hi_only:
            terms += [(wh_t, xl_t)]
        np_ = nk // 2
        n_ins = len(terms) * np_
        i = 0
        for (wt, xt) in terms:
            for j in range(np_):
                nc.tensor.matmul(
                    ps, wt[:, 2 * j:2 * j + 2, msl],
                    xt[:, 2 * j:2 * j + 2, :],
                    start=(start and i == 0),
                    stop=(stop and i == n_ins - 1),
                    perf_mode=DRow)
                i += 1

    def emit_rope(pool, dst64, src64, cos_ap, sin_ap, W, p0=0, ph=None):
        ph = ph or c.PEH
        t1, t2 = src64[0:ph], src64[ph:2 * ph]
        d1, d2 = dst64[0:ph], dst64[ph:2 * ph]
        c1, s1 = cos_ap[p0:p0 + ph], sin_ap[p0:p0 + ph]
        c2, s2 = cos_ap[p0 + ph:p0 + 2 * ph], sin_ap[p0 + ph:p0 + 2 * ph]
        ra = pool.tile([ph, W], BF, tag="rope_a", name="rope_a")
        rb = pool.tile([ph, W], BF, tag="rope_b", name="rope_b")
        nc.vector.tensor_tensor(out=ra, in0=t1, in1=c1, op=Alu.mult)
        nc.vector.tensor_tensor(out=rb, in0=t2, in1=s2, op=Alu.mult)
        nc.vector.tensor_tensor(out=d1, in0=ra, in1=rb, op=Alu.subtract)
        nc.vector.tensor_tensor(out=ra, in0=t2, in1=c2, op=Alu.mult)
        nc.vector.tensor_tensor(out=rb, in0=t1, in1=s1, op=Alu.mult)
        nc.vector.tensor_tensor(out=d2, in0=ra, in1=rb, op=Alu.add)

    with tile.TileContext(nc, pool_alloc_mode="queue") as tc:
        with contextlib.ExitStack() as top:
            pers = top.enter_context(tc.tile_pool(name="pers", bufs=1))
            cos_sb = pers.tile([128, c.S], BF, tag="cos_sb")
            sin_sb = pers.tile([128, c.S], BF, tag="sin_sb")
            cosa_sb = pers.tile([128, SL], BF, tag="cosa_sb")
            sina_sb = pers.tile([128, SL], BF, tag="sina_sb")
            lnq_sb = pers.tile([128, KT_Q], F32, tag="lnq_sb")
            lnkv_sb = pers.tile([128, KT_KV], F32, tag="lnkv_sb")
            mask_sb = pers.tile([128, TPC, c.CHUNK], BF, tag="mask_sb")
            ident = pers.tile([128, 128], BF, tag="ident")
            ones_f = pers.tile([1, 128], F32, tag="ones_f")
            ones_c = pers.tile([128, 1], BF, tag="ones_c")
            eps_sb = pers.tile([1, 1], F32, tag="eps_sb")
            nc.vector.memset(eps_sb, c.EPS)
            nc.sync.dma_start(out=cos_sb, in_=cosT.ap())
            nc.sync.dma_start(out=sin_sb, in_=sinT.ap())
            nc.sync.dma_start(out=cosa_sb, in_=cosA.ap())
            nc.sync.dma_start(out=sina_sb, in_=sinA.ap())
            nc.sync.dma_start(out=lnq_sb, in_=lnq.ap())
            nc.sync.dma_start(out=lnkv_sb, in_=lnkv.ap())
            nc.sync.dma_start(out=mask_sb, in_=maskm.ap())
            make_identity(nc, ident)
            nc.vector.memset(ones_f, 1.0)
            nc.vector.memset(ones_c, 1.0)

            # -------- phase A ------------------------------------------------
            with contextlib.ExitStack() as st:
                pax = st.enter_context(tc.tile_pool(name="pax", bufs=1))
                paw = st.enter_context(tc.tile_pool(name="paw", bufs=3))
                pat = st.enter_context(tc.tile_pool(name="pat", bufs=2))
                paa = st.enter_context(tc.tile_pool(name="paa", bufs=1))
                paps = st.enter_context(
                    tc.tile_pool(name="paps", bufs=3, space="PSUM"))
                pssq = st.enter_context(
                    tc.tile_pool(name="pssq", bufs=1, space="PSUM"))
                pbc = st.enter_context(
                    tc.tile_pool(name="pbc", bufs=2, space="PSUM"))

                xh_sb = pax.tile([128, KT_HID, SL], FP8, tag="xh_sb")
                xl_sb = pax.tile([128, KT_HID, SL], FP8, tag="xl_sb")
                nc.sync.dma_start(out=xh_sb, in_=xh_r)
                nc.sync.dma_start(out=xl_sb, in_=xl_r)
                aq_c = paa.tile([128, KT_Q, SL], BF, tag="aq_c")
                akv_c = paa.tile([128, KT_KV, SL], BF, tag="akv_c")
                aq8h = paa.tile([128, KT_Q, SL], FP8, tag="aq8h")
                aq8l = paa.tile([128, KT_Q, SL], FP8, tag="aq8l")
                akv8 = paa.tile([128, 2 * KT_KV + 2, SL], FP8, tag="akv8")
                ssq_q = pssq.tile([1, SL], F32, tag="ssq_q")
                ssq_kv = pssq.tile([1, SL], F32, tag="ssq_kv")
                kperaw = pat.tile([c.DR, SL], BF, tag="kperaw", bufs=1)
                kpel = pat.tile([c.DR, SL], BF, tag="kpel", bufs=1)

                def normalize(ssq, ln_sb, ktn, denom, dst):
                    rn = pat.tile([1, c.CHUNK], F32, tag="rn", name="rn")
                    nc.scalar.activation(
                        rn, ssq, Act.Sqrt, bias=eps_sb, scale=1.0 / denom)
                    rnr = pat.tile([1, c.CHUNK], F32, tag="rnr", name="rnr")
                    nc.vector.reciprocal(rnr, rn)
                    bc = pbc.tile([128, c.CHUNK], F32, tag="bc", name="bc")
                    nc.tensor.matmul(bc, ones_f, rnr, start=True, stop=True)
                    for t in range(ktn):
                        tgt = dst[:, t, :]
                        nc.vector.scalar_tensor_tensor(
                            out=tgt, in0=tgt, scalar=ln_sb[:, t:t + 1],
                            in1=bc, op0=Alu.mult, op1=Alu.mult)

                mtiles = ([("kv", i) for i in range(KT_KV)]
                          + [("pe", 0)]
                          + [("q", i) for i in range(KT_Q)])
                for mti, (seg, ti) in enumerate(mtiles):
                    mw = c.DR if seg == "pe" else 128
                    wah_sb = paw.tile([128, KT_HID, 128], FP8, tag="wah",
                                      name="wah")
                    wal_sb = paw.tile([128, KT_HID, 128], FP8, tag="wal",
                                      name="wal")
                    nc.sync.dma_start(
                        out=wah_sb,
                        in_=wa_h_r[:, mti].rearrange("p (k m) -> p k m",
                                                     m=128))
                    nc.sync.dma_start(
                        out=wal_sb,
                        in_=wa_l_r[:, mti].rearrange("p (k m) -> p k m",
                                                     m=128))
                    ps = paps.tile([128, c.CHUNK], F32, tag="aps", name="ps")
                    dr_terms(ps[:mw], wah_sb, wal_sb, xh_sb, xl_sb,
                             KT_HID, slice(0, mw))
                    if seg == "q":
                        nc.scalar.activation(aq_c[:, ti, :], ps, Act.Copy,
                                             scale=IV)
                        sq = pat.tile([128, c.CHUNK], BF, tag="sq", bufs=3,
                                      name="sq")
                        nc.scalar.activation(sq, ps, Act.Square, scale=IV)
                        nc.tensor.matmul(
                            ssq_q, ones_c, sq,
                            start=(ti == 0), stop=(ti == KT_Q - 1))
                    elif seg == "kv":
                        nc.scalar.activation(akv_c[:, ti, :], ps, Act.Copy,
                                             scale=IV)
                        sq = pat.tile([128, c.CHUNK], BF, tag="sq", bufs=3,
                                      name="sq")
                        nc.scalar.activation(sq, ps, Act.Square, scale=IV)
                        nc.tensor.matmul(
                            ssq_kv, ones_c, sq,
                            start=(ti == 0), stop=(ti == KT_KV - 1))
                    else:
                        nc.scalar.activation(kperaw, ps[:mw], Act.Copy,
                                             scale=IV)
                    if seg == "pe":
                        # kv part complete: normalize, rope, split, gather
                        normalize(ssq_kv, lnkv_sb, KT_KV, c.KVLR, akv_c)
                        emit_rope(pat, kpel, kperaw, cosa_sb, sina_sb,
                                  c.CHUNK)
                        for t in range(KT_KV):
                            nc.scalar.copy(akv8[:, t, :], akv_c[:, t, :])
                            nc.vector.scalar_tensor_tensor(
                                out=akv8[:, KT_KV + t, :],
                                in0=akv_c[:, t, :], scalar=1.0,
                                in1=akv8[:, t, :],
                                op0=Alu.mult, op1=Alu.subtract)
                        nc.scalar.copy(akv8[0:c.DR, 2 * KT_KV, :], kpel)
                        nc.vector.scalar_tensor_tensor(
                            out=akv8[0:c.DR, 2 * KT_KV + 1, :],
                            in0=kpel, scalar=1.0,
                            in1=akv8[0:c.DR, 2 * KT_KV, :],
                            op0=Alu.mult, op1=Alu.subtract)
                        nc.sync.dma_start(out=aglkv_r, in_=akv8)
                        if GS > 1:
                            nc.gpsimd.collective_compute(
                                "AllGather", Alu.bypass,
                                replica_groups=groups,
                                ins=[aglkv.ap()], outs=[aggkv.ap()])
                normalize(ssq_q, lnq_sb, KT_Q, c.QLR, aq_c)
                for t in range(KT_Q):
                    nc.scalar.copy(aq8h[:, t, :], aq_c[:, t, :])
                    nc.vector.scalar_tensor_tensor(
                        out=aq8l[:, t, :], in0=aq_c[:, t, :], scalar=1.0,
                        in1=aq8h[:, t, :], op0=Alu.mult, op1=Alu.subtract)
                nc.sync.dma_start(out=aglq_r[:, 0:KT_Q, :], in_=aq8h)
                nc.sync.dma_start(out=aglq_r[:, KT_Q:2 * KT_Q, :], in_=aq8l)
                if GS > 1:
                    nc.gpsimd.collective_compute(
                        "AllGather", Alu.bypass, replica_groups=groups,
                        ins=[aglq.ap()], outs=[aggq.ap()])

            # -------- phase B: kv up-projection ------------------------------
            if "B" in enabled:
                pkv = top.enter_context(tc.tile_pool(name="pkv", bufs=1))
                kn = pkv.tile([128, H, 2, c.S], FP8, tag="kn")
                vv = pkv.tile([128, ST, H, c.DV + 1], BF, tag="vv")
                nc.vector.memset(vv[:, :, :, c.DV:], 1.0 / OS)
                kpe8 = pkv.tile([c.DR, 2, c.S], FP8, tag="kpe8")
                for g in range(GS):
                    nc.sync.dma_start(
                        out=kpe8[:, :, g * SL:(g + 1) * SL],
                        in_=aggkv_r[g, 0:c.DR, 2 * KT_KV:2 * KT_KV + 2, :])

                with contextlib.ExitStack() as st:
                    pbw = st.enter_context(tc.tile_pool(name="pbw", bufs=1))
                    pbps = st.enter_context(
                        tc.tile_pool(name="pbps", bufs=3, space="PSUM"))
                    wkvh_sb = pbw.tile([128, KT_KV, KROWS + VCOLS], FP8,
                                       tag="wkvh")
                    wkvl_sb = pbw.tile([128, KT_KV, KROWS + VCOLS], FP8,
                                       tag="wkvl")
                    nc.sync.dma_start(out=wkvh_sb, in_=wkv_h_r)
                    nc.sync.dma_start(out=wkvl_sb, in_=wkv_l_r)
                    akvfh = pbw.tile([128, KT_KV, c.S], FP8, tag="akvfh")
                    akvfl = pbw.tile([128, KT_KV, c.S], FP8, tag="akvfl")
                    for g in range(GS):
                        sl_ = slice(g * SL, (g + 1) * SL)
                        nc.sync.dma_start(out=akvfh[:, :, sl_],
                                          in_=aggkv_r[g, :, 0:KT_KV, :])
                        nc.sync.dma_start(out=akvfl[:, :, sl_],
                                          in_=aggkv_r[g, :, KT_KV:2 * KT_KV, :])
                    ev_flip = [0]

                    def evict(dst, src, scale):
                        # alternate Act / DVE to balance engine load
                        if ev_flip[0] % 2 == 0:
                            nc.scalar.activation(dst, src, Act.Copy,
                                                 scale=scale)
                        else:
                            nc.vector.tensor_scalar_mul(dst, src, scale)
                        ev_flip[0] += 1

                    for mt in range(H):
                        msl = slice(mt * 128, (mt + 1) * 128)
                        for qc in range(NQC):
                            cs = slice(qc * c.CHUNK, (qc + 1) * c.CHUNK)
                            ps = pbps.tile([128, c.CHUNK], F32, tag="kps")
                            dr_terms(ps, wkvh_sb, wkvl_sb,
                                     akvfh[:, :, cs], akvfl[:, :, cs],
                                     KT_KV, msl)
                            nc.scalar.activation(kn[:, mt, 0, cs], ps,
                                                 Act.Copy, scale=IV)
                            nc.vector.scalar_tensor_tensor(
                                out=kn[:, mt, 1, cs], in0=ps, scalar=IV,
                                in1=kn[:, mt, 0, cs],
                                op0=Alu.mult, op1=Alu.subtract)
                    vch = []
                    v0 = 0
                    while v0 < VCOLS:
                        vw = min(512, VCOLS - v0)
                        vch.append((v0, vw))
                        v0 += vw
                    for stt_ in range(ST):
                        ss = slice(stt_ * 128, (stt_ + 1) * 128)
                        for v0, vw in vch:
                            vsl = slice(KROWS + v0, KROWS + v0 + vw)
                            ps = pbps.tile([128, 512], F32, tag="vps")
                            dr_terms(ps[:, :vw], akvfh[:, :, ss],
                                     akvfl[:, :, ss], wkvh_sb[:, :, vsl],
                                     wkvl_sb[:, :, vsl], KT_KV,
                                     slice(0, 128))
                            h0, hn = v0 // c.DV, vw // c.DV
                            evict(
                                vv[:, stt_, h0:h0 + hn, 0:c.DV],
                                ps[:, :vw].rearrange("p (h d) -> p h d",
                                                     d=c.DV), IV)

            # -------- phase C: q up-projection (single comp3 pass) -----------
            if "C" in enabled:
                with contextlib.ExitStack() as st:
                    pcq = st.enter_context(tc.tile_pool(name="pcq", bufs=1))
                    pcw = st.enter_context(tc.tile_pool(name="pcw", bufs=3))
                    pce = st.enter_context(tc.tile_pool(name="pce", bufs=3))
                    pcps = st.enter_context(
                        tc.tile_pool(name="pcps", bufs=4, space="PSUM"))
                    aqfh = pcq.tile([128, KT_Q, c.S], FP8, tag="aqfh")
                    aqfl = pcq.tile([128, KT_Q, c.S], FP8, tag="aqfl")
                    for g in range(GS):
                        sl_ = slice(g * SL, (g + 1) * SL)
                        nc.sync.dma_start(out=aqfh[:, :, sl_],
                                          in_=aggq_r[g, :, 0:KT_Q, :])
                        nc.sync.dma_start(out=aqfl[:, :, sl_],
                                          in_=aggq_r[g, :, KT_Q:2 * KT_Q, :])
                    mt_order = []
                    for j in range(MT_QP):
                        mt_order += [2 * j, 2 * j + 1, MT_QN + j]
                    mt_order += list(range(2 * MT_QP, MT_QN))
                    for mt in mt_order:
                        wq_sb = pcw.tile([128, 2, KT_Q, 128], FP8, tag="wq")
                        nc.sync.dma_start(
                            out=wq_sb,
                            in_=wqb_hl_r[:, mt].rearrange(
                                "p (t k m) -> p t k m", t=2, m=128))
                        for qc in range(NQC):
                            col = qc * c.CHUNK
                            cs = slice(col, col + c.CHUNK)
                            ps = pcps.tile([128, c.CHUNK], F32, tag="qps")
                            dr_terms(ps, wq_sb[:, 0], wq_sb[:, 1],
                                     aqfh[:, :, cs], aqfl[:, :, cs],
                                     KT_Q, slice(0, 128))
                            if mt >= MT_QN:
                                qsb = pce.tile([128, c.CHUNK], BF, tag="qsb")
                                nc.scalar.activation(qsb, ps, Act.Copy,
                                                     scale=IV)
                                roped = pce.tile([128, c.CHUNK], FP8,
                                                 tag="roped")
                                emit_rope(pce, roped, qsb,
                                          cos_sb[:, cs], sin_sb[:, cs],
                                          c.CHUNK, ph=64)
                                j2 = mt - MT_QN
                                for sl8 in (256, 384):
                                    for j in (0, 1):
                                        dst = qTs8_r[
                                            2 * j2:2 * j2 + 2,
                                            sl8 + j * 32:sl8 + j * 32 + 32,
                                            cs]
                                        nc.sync.dma_start(
                                            out=dst,
                                            in_=roped[j * 64:(j + 1) * 64])
                            else:
                                qf8 = pce.tile([128, c.CHUNK], FP8,
                                               tag="qf8")
                                nc.scalar.activation(qf8, ps, Act.Copy,
                                                     scale=IV)
                                dst = qTs8_r[mt, 0:256, cs].rearrange(
                                    "(two p) s -> p two s", p=128)
                                nc.sync.dma_start(
                                    out=dst,
                                    in_=qf8[:, None, :].broadcast_to(
                                        [128, 2, c.CHUNK]))
            # -------- phase D: attention -------------------------------------
            if "D" in enabled:
                pot = top.enter_context(tc.tile_pool(name="pot", bufs=1))
                oT_h = pot.tile([128, H, c.S], FP8, tag="oT_h")
                oT_l = pot.tile([128, H, c.S], FP8, tag="oT_l")

                with contextlib.ExitStack() as st:
                    pdp = st.enter_context(tc.tile_pool(name="pdp", bufs=2))
                    pdq = st.enter_context(tc.tile_pool(name="pdq", bufs=3))
                    pde = st.enter_context(tc.tile_pool(name="pde", bufs=6))
                    pew = st.enter_context(tc.tile_pool(name="pew", bufs=3))
                    peo = st.enter_context(tc.tile_pool(name="peo", bufs=3))
                    pds = st.enter_context(
                        tc.tile_pool(name="pds", bufs=2, space="PSUM"))
                    pdo = st.enter_context(
                        tc.tile_pool(name="pdo", bufs=1, space="PSUM"))
                    pdt = st.enter_context(
                        tc.tile_pool(name="pdt", bufs=1, space="PSUM"))
                    peps = st.enter_context(
                        tc.tile_pool(name="peps", bufs=2, space="PSUM"))

                    def e_tiles(qc, mts):
                        cs = slice(qc * c.CHUNK, (qc + 1) * c.CHUNK)
                        for mt in mts:
                            wo_sb = pew.tile([128, 2, H, 128], FP8, tag="wo")
                            nc.sync.dma_start(
                                out=wo_sb,
                                in_=wo_hl_r[:, mt].rearrange(
                                    "p (t k m) -> p t k m", t=2, m=128))
                            ps = peps.tile([128, c.CHUNK], F32, tag="ops")
                            dr_terms(ps, wo_sb[:, 0], wo_sb[:, 1],
                                     oT_h[:, :, cs],
                                     oT_l[:, :, cs], H, slice(0, 128))
                            ob = peo.tile([128, c.CHUNK], F32, tag="ob")
                            if mt % 2 == 0:
                                nc.scalar.copy(ob, ps)
                            else:
                                nc.vector.tensor_copy(out=ob, in_=ps)
                            nc.sync.dma_start(
                                out=outT_ap[mt * 128:(mt + 1) * 128, cs],
                                in_=ob)

                    def d_chunk(qc, eq):
                        col = qc * c.CHUNK
                        kmax = min(TPC * qc + TPC, ST)
                        for h in range(H):
                            qf = pdq.tile([128, 4, c.CHUNK], FP8, tag="qf")
                            nc.sync.dma_start(
                                out=qf,
                                in_=qTs8_r[h, :, col:col + c.CHUNK].rearrange(
                                    "(fo p) s -> p fo s", p=128))
                            probs = pdp.tile([128, ST, c.CHUNK], BF,
                                             tag="probs")
                            for k2 in range(kmax // 2):
                                ps = pds.tile([128, 2, c.CHUNK], F32,
                                              tag="sc")
                                for i in (0, 1):
                                    kt = 2 * k2 + i
                                    ksl = slice(kt * 128, (kt + 1) * 128)
                                    nc.tensor.matmul(
                                        ps[:, i, :], kn[:, h, :, ksl],
                                        qf[:, 0:2, :], start=True,
                                        stop=False, perf_mode=DRow)
                                    nc.tensor.matmul(
                                        ps[:, i, :], kpe8[:, :, ksl],
                                        qf[0:64, 2:4, :], start=False,
                                        stop=True, perf_mode=DRow)
                                nc.scalar.activation(
                                    probs[:, 2 * k2:2 * k2 + 2, :], ps,
                                    Act.Exp, scale=SCALE)
                                for i in (0, 1):
                                    kt = 2 * k2 + i
                                    d = kt - TPC * qc
                                    if d >= 0:
                                        nc.vector.tensor_tensor(
                                            out=probs[:, kt, :],
                                            in0=probs[:, kt, :],
                                            in1=mask_sb[:, d, :],
                                            op=Alu.mult)
                            for q2 in range(TPC):
                                qt = TPC * qc + q2
                                po = pdo.tile([128, c.DV + 1], F32, tag="po")
                                for kt in range(qt + 1):
                                    nc.tensor.matmul(
                                        po,
                                        probs[:, kt, q2 * 128:(q2 + 1) * 128],
                                        vv[:, kt, h, :],
                                        start=(kt == 0), stop=(kt == qt))
                                rec = pde.tile([128, 1], F32, tag="rec")
                                nc.vector.reciprocal(
                                    rec, po[:, c.DV:c.DV + 1])
                                osb = pde.tile([128, c.DV], BF, tag="osb")
                                nc.vector.tensor_scalar_mul(
                                    osb, po[:, :c.DV], rec)
                                pt = pdt.tile([128, 128], BF, tag="pt")
                                nc.tensor.transpose(pt, osb, ident)
                                ql_ = slice(qt * 128, (qt + 1) * 128)
                                nc.vector.tensor_copy(
                                    out=oT_h[:, h, ql_], in_=pt)
                                nc.vector.scalar_tensor_tensor(
                                    out=oT_l[:, h, ql_], in0=pt, scalar=1.0,
                                    in1=oT_h[:, h, ql_],
                                    op0=Alu.mult, op1=Alu.subtract)
                            if eq is not None and "E" in enabled:
                                e_tiles(eq, range(4 * h, 4 * h + 4))

                    d_chunk(0, None)
                    d_chunk(1, 0)
                    d_chunk(2, 1)
                    d_chunk(3, 2)
                    if "E" in enabled:
                        e_tiles(3, range(MT_O))

    nc.compile()
    return nc


# ---------------------------------------------------------------------------
# host-side input preparation
# ---------------------------------------------------------------------------

def split8(x):
    hi = np.asarray(x, np.float32).astype(E4M3)
    lo = (np.asarray(x, np.float32) - hi.astype(np.float32)).astype(E4M3)
    return hi, lo


def prep_shared(c: Cfg, w_a, q_ln_w, kv_ln_w):
    KT_Q = c.QLR // 128
    KT_KV = c.KVLR // 128
    TPC = c.CHUNK // 128
    half = c.PEH
    inv_freq = 1.0 / (c.THETA ** (np.arange(half, dtype=np.float32) / half))
    ang = np.arange(c.S, dtype=np.float32)[:, None] * inv_freq[None, :]
    cosT = np.ascontiguousarray(
        np.tile(np.cos(ang).T, (128 // half, 1))).astype(BF16)
    sinT = np.ascontiguousarray(
        np.tile(np.sin(ang).T, (128 // half, 1))).astype(BF16)
    k_idx = np.arange(128)[:, None]
    q_idx = np.arange(c.CHUNK)[None, :]
    maskm = np.stack(
        [(k_idx <= q_idx - 128 * d) for d in range(TPC)], axis=1
    ).astype(BF16)
    MT_A = KT_Q + KT_KV + 1
    KT_HID = c.HID // 128
    wa = np.asarray(w_a, np.float32) * WS
    order = ([c.QLR + i * 128 for i in range(KT_KV)]
             + [c.QLR + c.KVLR]
             + [i * 128 for i in range(KT_Q)])
    tiles = []
    for m0 in order:
        t = np.zeros((c.HID, 128), np.float32)
        wsrc = wa[:, m0:m0 + 128]
        t[:, :wsrc.shape[1]] = wsrc
        tiles.append(t)
    wa_t = np.stack(tiles, axis=1)  # [HID, MT_A, 128]
    wa_t = wa_t.reshape(KT_HID, 128, MT_A, 128).transpose(1, 2, 0, 3)
    wa_t = np.ascontiguousarray(wa_t.reshape(128, MT_A * KT_HID * 128))
    wa_hi, wa_lo = split8(wa_t)
    return {
        "wa_h": wa_hi, "wa_l": wa_lo,
        "lnq": np.ascontiguousarray(
            np.asarray(q_ln_w).reshape(KT_Q, 128).T).astype(np.float32),
        "lnkv": np.ascontiguousarray(
            np.asarray(kv_ln_w).reshape(KT_KV, 128).T).astype(np.float32),
        "cosT": cosT,
        "sinT": sinT,
        "maskm": np.ascontiguousarray(maskm),
    }


def prep_group(c: Cfg, heads, w_qb, w_kvb, w_o, n_heads_total):
    wq = np.asarray(w_qb, np.float32).reshape(
        c.QLR, n_heads_total, c.DQK)[:, heads, :] * WS
    H_ = c.HPC
    # qp columns permuted per head-pair to [t, h2, 32] so rope runs on
    # 64-row halves; un-permuted by the qTs write AP in the kernel
    wq_pe = wq[:, :, c.DN:].reshape(c.QLR, H_ // 2, 2, 2, 32)
    wq_pe = wq_pe.transpose(0, 1, 3, 2, 4).reshape(c.QLR, -1)
    wq_g = np.concatenate(
        [wq[:, :, :c.DN].reshape(c.QLR, -1), wq_pe], axis=1)
    wkv = np.asarray(w_kvb, np.float32).reshape(
        c.KVLR, n_heads_total, c.DN + c.DV)[:, heads, :] * WS
    wkv_g = np.concatenate(
        [wkv[:, :, :c.DN].reshape(c.KVLR, -1),
         wkv[:, :, c.DN:].reshape(c.KVLR, -1)], axis=1)
    wo_g = np.asarray(w_o, np.float32).reshape(
        n_heads_total, c.DV, c.HID)[heads].reshape(-1, c.HID) * WS
    H = c.HPC
    KT_Q = c.QLR // 128
    KT_KV = c.KVLR // 128
    MT_QB = wq_g.shape[1] // 128
    MT_O = c.HID // 128
    wq_t = wq_g.reshape(KT_Q, 128, MT_QB, 128).transpose(1, 2, 0, 3)
    wq_t = np.ascontiguousarray(wq_t.reshape(128, MT_QB * KT_Q * 128))
    wkv_t = wkv_g.reshape(KT_KV, 128, wkv_g.shape[1]).transpose(1, 0, 2)
    wkv_t = np.ascontiguousarray(wkv_t.reshape(128, -1))
    wo_t = wo_g.reshape(H, 128, MT_O, 128).transpose(1, 2, 0, 3)
    wo_t = np.ascontiguousarray(wo_t.reshape(128, MT_O * H * 128))
    wq_h, wq_l = split8(wq_t)
    wkv_hi, wkv_lo = split8(wkv_t)
    wo_hi, wo_lo = split8(wo_t)
    MT_QB2 = MT_QB
    wq_hl = np.stack([wq_h.reshape(128, MT_QB2, KT_Q * 128),
                      wq_l.reshape(128, MT_QB2, KT_Q * 128)], axis=2)
    wo_hl = np.stack([wo_hi.reshape(128, MT_O, H * 128),
                      wo_lo.reshape(128, MT_O, H * 128)], axis=2)
    return {
        "wqb_hl": np.ascontiguousarray(wq_hl.reshape(128, -1)),
        "wkv_h": wkv_hi, "wkv_l": wkv_lo,
        "wo_hl": np.ascontiguousarray(wo_hl.reshape(128, -1)),
    }


_PROGRAM = None


def _get_program():
    global _PROGRAM
    if _PROGRAM is None:
        _PROGRAM = build_program(FULL)
    return _PROGRAM


def kernel(x, w_a, q_ln_w, kv_ln_w, w_qb, w_kvb, w_o):
    from concourse.bass_utils import run_bass_kernel_spmd

    c = FULL
    x = np.asarray(x, dtype=np.float32)
    B = x.shape[0]
    n_heads = w_qb.shape[1] // c.DQK
    n_groups = n_heads // c.HPC
    assert B * n_groups == c.NCORES and n_groups == c.GS

    nc = _get_program()
    shared = prep_shared(c, np.asarray(w_a), np.asarray(q_ln_w),
                         np.asarray(kv_ln_w))
    groups = [
        prep_group(c, slice(g * c.HPC, (g + 1) * c.HPC), np.asarray(w_qb),
                   np.asarray(w_kvb), np.asarray(w_o), n_heads)
        for g in range(n_groups)
    ]

    in_maps = []
    for core in range(c.NCORES):
        b, g = divmod(core, n_groups)
        sl = slice(g * c.SL, (g + 1) * c.SL)
        xtl = np.ascontiguousarray(x[b].T[:, sl]).reshape(
            c.HID // 128, 128, c.SL)
        xtl = np.ascontiguousarray(xtl.transpose(1, 0, 2).reshape(128, -1))
        xtl_h, xtl_l = split8(xtl)
        in_maps.append({
            "xh": xtl_h, "xl": xtl_l,
            "cosA": np.ascontiguousarray(shared["cosT"][:, sl]),
            "sinA": np.ascontiguousarray(shared["sinT"][:, sl]),
            **shared, **groups[g],
        })

    res = run_bass_kernel_spmd(nc, in_maps, core_ids=list(range(c.NCORES)))
    outs = [r["outT"] for r in res.results]
    result = np.empty((B, c.S, c.HID), dtype=np.float32)
    inv = 1.0 / (WS * OS)
    for b in range(B):
        acc = outs[b * n_groups].copy()
        for g in range(1, n_groups):
            acc += outs[b * n_groups + g]
        result[b] = acc.T * inv
    return result
